# revision 1
# baseline (speedup 1.0000x reference)
"""GQA attention kernel for Trainium2, sharded over 8 NeuronCores.

Sharding: tensor-parallel over heads. Core c owns kv-head c and q-heads
4c..4c+3 (rows 256c:256c+256 of Wq, rows 64c:64c+64 of Wk/Wv) and columns
256c:256c+256 of Wo. Each core computes a full-shape partial of the output
(o_proj column-parallel); the host sums the 8 partials (the all-reduce)
and adds bo.

Per-core kernel layout choices:
- hidden_states is passed transposed [H, B*S] so QKV projections contract
  over the partition dim with contiguous DMA.
- Q,K,V are produced transposed ([feature, token]) directly from the PE.
- Scores are computed transposed, S^T[t, q] = K_d,t^T . Q_d,q, so the
  softmax mask+scale fold into the exp activation (mask is per-partition),
  and a ones-column appended to V yields softmax denominators as row 64 of
  the context matmul output.
- All matmuls use float32r (full-rate fp32 on TRN2 when N >= 256).
"""

import os
import sys

for _p in ("/opt/trn_rl_repo",):
    if _p not in sys.path and os.path.isdir(_p):
        sys.path.insert(0, _p)

import numpy as np

import concourse.bass as bass
import concourse.bacc as bacc
import concourse.tile as tile
from concourse import mybir
from concourse import bass_utils

F32 = mybir.dt.float32
F32R = mybir.dt.float32r
AF = mybir.ActivationFunctionType

B = 2
S = 2048
H = 2048
D = 64
N_CORES = 8
QH_PER_CORE = 4          # q-heads per core
QF = QH_PER_CORE * D     # 256 q features per core
TOK = B * S              # 4096
SCALE = 1.0 / np.sqrt(D)  # 0.125

_CACHE = {}


def _build_program():
    nc = bacc.Bacc("TRN2", target_bir_lowering=False, debug=False)

    hsT = nc.dram_tensor("hsT", [H, TOK], F32R, kind="ExternalInput").ap()
    wqkvT = nc.dram_tensor("wqkvT", [H, 384], F32R, kind="ExternalInput").ap()
    woT = nc.dram_tensor("woT", [QF, H], F32R, kind="ExternalInput").ap()
    bqkv = nc.dram_tensor("bqkv", [128, 3], F32, kind="ExternalInput").ap()
    maskp = nc.dram_tensor("maskp", [128, B, S // 128], F32, kind="ExternalInput").ap()
    eye = nc.dram_tensor("eye", [128, 64], F32R, kind="ExternalInput").ap()
    out = nc.dram_tensor("out", [B, S, H], F32, kind="ExternalOutput").ap()

    with tile.TileContext(nc) as tc:
        with tc.tile_pool(name="const", bufs=1) as cp:
            w_qkv = cp.tile([128, 16, 384], F32R)     # (p, h_tile, feature)
            nc.sync.dma_start(out=w_qkv, in_=wqkvT.rearrange("(t p) f -> p t f", p=128))
            w_o = cp.tile([128, 2, H], F32R)          # (p, f_tile, e)
            nc.sync.dma_start(out=w_o, in_=woT.rearrange("(t p) e -> p t e", p=128))
            bqkv_sb = cp.tile([128, 3], F32)
            nc.sync.dma_start(out=bqkv_sb, in_=bqkv)
            mask_sb = cp.tile([128, B, S // 128], F32)
            nc.sync.dma_start(out=mask_sb, in_=maskp)
            eye_sb = cp.tile([128, 64], F32R)
            nc.sync.dma_start(out=eye_sb, in_=eye)

            # Engine wait budgets are tiny (1 sync-wait per instruction for
            # PE/ACT structs). Warm each consumer engine's vector clock on the
            # small const DMAs so real instructions never need a second wait.
            scratch = cp.tile([128, 1], F32)
            nc.scalar.copy(out=scratch, in_=bqkv_sb[:, 0:1])
            nc.scalar.copy(out=scratch, in_=mask_sb[:, 0, 0:1])

            # Q^T, K^T, V^T resident in SBUF: qkvT[0] = q feats 0:128,
            # qkvT[1] = q feats 128:256, qkvT[2] = [K (0:64) | V (64:128)].
            qkvT = [cp.tile([128, TOK], F32R, name=f"qkvT{i}") for i in range(3)]
            # V transposed back to [t, d] + ones column, per 128-token tile.
            vones = cp.tile([128, B * 16, 65], F32R)
            # K^T replicated in both partition halves so each q-head's scores
            # matmul finds K at its own base partition (PE alignment rule).
            k2 = cp.tile([128, TOK], F32R)

            # ---- QKV projections ----
            hsT_tiled = hsT.rearrange("(t p) n -> p t n", p=128)
            with tc.tile_pool(name="proj_sb", bufs=2) as psb, \
                 tc.tile_pool(name="proj_ps", bufs=3, space="PSUM") as pps:
                CK = 256
                # fp32r matmuls encode a single sync-wait slot. A dummy [1,1]
                # matmul "spends" one DMA wait on the PE clock so the first
                # real matmul of each chunk only needs its remaining wait.
                dps = pps.tile([1, 1], F32, tag="dummy", bufs=1)
                nc.tensor.matmul(dps, w_o[:, 0, 0:1].bitcast(F32), w_o[:, 0, 0:1].bitcast(F32),
                                 start=True, stop=True)
                for ck in range(TOK // CK):
                    hstage = psb.tile([128, 16, CK], F32R, tag="hstage")
                    # 16 piece-DMAs: queue round-robin makes slot reuse land
                    # on the same queue (implicit WAW), and each consuming
                    # matmul carries exactly one piece-wait.
                    for ht in range(16):
                        nc.sync.dma_start(
                            out=hstage[:, ht, :],
                            in_=hsT_tiled[:, ht, ck * CK:(ck + 1) * CK])
                    nc.tensor.matmul(dps, hstage[:, 0, 0:1].bitcast(F32), hstage[:, 0, 0:1].bitcast(F32),
                                     start=True, stop=True)
                    for ft in range(3):
                        ps = pps.tile([128, CK], F32, tag="projps", bufs=3)
                        for ht in range(16):
                            nc.tensor.matmul(
                                ps,
                                w_qkv[:, ht, ft * 128:(ft + 1) * 128],
                                hstage[:, ht, :],
                                start=(ht == 0), stop=(ht == 15),
                            )
                        nc.scalar.activation(
                            out=qkvT[ft][:, ck * CK:(ck + 1) * CK], in_=ps,
                            func=AF.Identity, bias=bqkv_sb[:, ft:ft + 1],
                        )
                # ---- build V[t, d] (+ones) from V^T via PE transpose ----
                for bt in range(B * 16):
                    tp = pps.tile([128, 64], F32R, tag="vtrans", bufs=2)
                    nc.tensor.transpose(
                        tp, in_=qkvT[2][64:128, bt * 128:(bt + 1) * 128],
                        identity=eye_sb[64:128, :])
                    nc.scalar.copy(out=vones[:, bt, 0:64], in_=tp)
                # ones column via ACT (keeps vones single-writer-proc: ACT
                # only), computed as 0*mask + 1 from a known-finite input.
                nc.scalar.activation(
                    out=vones[:, :, 64:65],
                    in_=mask_sb.rearrange("p b t -> p (b t)"),
                    func=AF.Identity, bias=1.0, scale=0.0)
                nc.sync.dma_start(out=k2[0:64, :], in_=qkvT[2][0:64, :])
                nc.sync.dma_start(out=k2[64:128, :], in_=qkvT[2][0:64, :])
                nc.tensor.matmul(dps, k2[0:64, 0:1].bitcast(F32), k2[0:64, 0:1].bitcast(F32),
                                 start=True, stop=True)
                nc.tensor.matmul(dps, k2[64:128, 0:1].bitcast(F32), k2[64:128, 0:1].bitcast(F32),
                                 start=True, stop=True)

            # ---- attention + o_proj ----
            with tc.tile_pool(name="att_sb", bufs=3) as asb, \
                 tc.tile_pool(name="drain_sb", bufs=3) as dsb, \
                 tc.tile_pool(name="ctxT_sb", bufs=2) as csb, \
                 tc.tile_pool(name="scores_ps", bufs=2, space="PSUM") as sps, \
                 tc.tile_pool(name="ctx_ps", bufs=2, space="PSUM") as xps, \
                 tc.tile_pool(name="o_ps", bufs=2, space="PSUM") as ops_pool:
                for b in range(B):
                    for qh in range(2):          # 1024-token q chunks
                        q0 = b * S + qh * 1024
                        ctxT = [csb.tile([128, 1024], F32R, tag=f"ctxT{ft}",
                                         name=f"ctxT{ft}_{b}_{qh}") for ft in range(2)]
                        # pre-spend the ctxT slot-reuse wait (PE o_proj
                        # release) on DVE before the first normalize write
                        for ft in range(2):
                            nc.vector.memset(ctxT[ft][0:1, 0:1].bitcast(F32), 0.0)
                        for g in range(QH_PER_CORE):
                            qt = qkvT[g // 2]
                            qp = (g % 2) * 64
                            ctx0 = xps.tile([65, 512], F32, tag="ctx")
                            ctx1 = xps.tile([65, 512], F32, tag="ctx")
                            ctxs = (ctx0, ctx1)
                            # wait-carrier: spend the ctx-slot WAR wait (DVE
                            # release) before the real t=0 accumulation start.
                            nc.tensor.matmul(ctx0[0:1, 0:1], w_qkv[:, 0, 0:1].bitcast(F32),
                                             w_qkv[:, 0, 0:1].bitcast(F32), start=True, stop=True)
                            nc.tensor.matmul(ctx1[0:1, 0:1], w_qkv[:, 0, 0:1].bitcast(F32),
                                             w_qkv[:, 0, 0:1].bitcast(F32), start=True, stop=True)
                            for t in range(16):
                                sc = sps.tile([128, 1024], F32, tag="scores")
                                for qc in range(2):
                                    nc.tensor.matmul(
                                        sc[:, qc * 512:(qc + 1) * 512],
                                        k2[qp:qp + 64, b * S + t * 128:b * S + (t + 1) * 128],
                                        qt[qp:qp + 64, q0 + qc * 512:q0 + (qc + 1) * 512],
                                        start=True, stop=True,
                                    )
                                ex = asb.tile([128, 1024], F32R, tag="expT")
                                nc.scalar.activation(
                                    out=ex, in_=sc, func=AF.Exp,
                                    bias=mask_sb[:, b, t:t + 1], scale=SCALE,
                                )
                                for qc in range(2):
                                    nc.tensor.matmul(
                                        ctxs[qc],
                                        vones[:, b * 16 + t, :],
                                        ex[:, qc * 512:(qc + 1) * 512],
                                        start=(t == 0), stop=(t == 15),
                                    )
                            # drain: copy out of PSUM, normalize by row 64
                            for qc in range(2):
                                cs = dsb.tile([65, 512], F32, tag="ctx_sb")
                                nc.vector.tensor_copy(out=cs, in_=ctxs[qc])
                                rc = dsb.tile([1, 512], F32, tag="recip")
                                nc.vector.reciprocal(out=rc, in_=cs[64:65, :])
                                bc = dsb.tile([64, 512], F32, tag="bcast")
                                nc.gpsimd.partition_broadcast(bc, rc)
                                nc.vector.tensor_mul(
                                    out=ctxT[g // 2][qp:qp + 64, qc * 512:(qc + 1) * 512],
                                    in0=cs[0:64, :], in1=bc,
                                )
                        # o_proj for this (b, qh): out[tok, e] partial
                        for qq in range(8):
                            osb = asb.tile([128, H], F32, tag="osb", name=f"osb_{b}_{qh}_{qq}")
                            # pre-spend the osb slot-reuse wait (out-DMA done)
                            nc.vector.memset(osb[0:1, 0:1], 0.0)
                            for ec in range(4):
                                op = ops_pool.tile([128, 512], F32, tag="ops")
                                for ft in range(2):
                                    nc.tensor.matmul(
                                        op,
                                        ctxT[ft][:, qq * 128:(qq + 1) * 128],
                                        w_o[:, ft, ec * 512:(ec + 1) * 512],
                                        start=(ft == 0), stop=(ft == 1),
                                    )
                                nc.vector.tensor_copy(
                                    out=osb[:, ec * 512:(ec + 1) * 512], in_=op)
                            nc.sync.dma_start(
                                out=out[b, qh * 1024 + qq * 128:qh * 1024 + (qq + 1) * 128, :],
                                in_=osb,
                            )
    nc.compile()
    return nc


def kernel(hidden_states, attention_mask, Wq, bq, Wk, bk, Wv, bv, Wo, bo):
    hidden_states = np.asarray(hidden_states, dtype=np.float32)
    attention_mask = np.asarray(attention_mask, dtype=np.float32)
    Wq = np.asarray(Wq, dtype=np.float32)
    Wk = np.asarray(Wk, dtype=np.float32)
    Wv = np.asarray(Wv, dtype=np.float32)
    Wo = np.asarray(Wo, dtype=np.float32)

    if "nc" not in _CACHE:
        _CACHE["nc"] = _build_program()
    nc = _CACHE["nc"]

    hsT = np.ascontiguousarray(
        hidden_states.reshape(TOK, H).T)                      # [H, B*S]
    maskp = np.ascontiguousarray(
        attention_mask.reshape(B, S // 128, 128).transpose(2, 0, 1))  # [128, B, 16]
    eye = np.zeros((128, 64), dtype=np.float32)
    eye[64:128, :] = np.eye(64, dtype=np.float32)

    in_maps = []
    for c in range(N_CORES):
        wq = Wq[QF * c:QF * (c + 1)]          # [256, H]
        wk = Wk[D * c:D * (c + 1)]            # [64, H]
        wv = Wv[D * c:D * (c + 1)]            # [64, H]
        wqkvT = np.ascontiguousarray(np.concatenate([wq, wk, wv], axis=0).T)  # [H, 384]
        woT = np.ascontiguousarray(Wo[:, QF * c:QF * (c + 1)].T)              # [256, H]
        bqkv = np.ascontiguousarray(
            np.concatenate([bq[QF * c:QF * (c + 1)], bk[D * c:D * (c + 1)],
                            bv[D * c:D * (c + 1)]]).astype(np.float32)
            .reshape(3, 128).T)               # [128, 3]
        in_maps.append({
            "hsT": hsT, "wqkvT": wqkvT, "woT": woT,
            "bqkv": bqkv, "maskp": maskp, "eye": eye,
        })

    _CACHE["last_in_maps"] = in_maps
    res = bass_utils.run_bass_kernel_spmd(nc, in_maps, core_ids=list(range(N_CORES)))
    acc = np.zeros((B, S, H), dtype=np.float32)
    for c in range(N_CORES):
        acc += res.results[c]["out"]
    acc += np.asarray(bo, dtype=np.float32)[None, None, :]
    return acc



# revision 32
# speedup vs baseline: 1.3813x; 1.3813x over previous
"""GQA attention kernel for Trainium2, sharded over 8 NeuronCores.

Sharding: tensor-parallel over heads. Core c owns kv-head c and q-heads
4c..4c+3 (rows 256c:256c+256 of Wq, rows 64c:64c+64 of Wk/Wv) and columns
256c:256c+256 of Wo. Each core computes a full-shape partial of the output
(o_proj column-parallel); the host sums the 8 partials (the all-reduce)
and adds bo.

Per-core kernel layout choices:
- hidden_states is passed transposed [H, B*S] in bf16 so QKV projections
  contract over the partition dim with one big contiguous DMA per 256-token
  chunk (descriptor-generation time is per-DMA).
- Q,K,V are produced transposed ([feature, token]) by the PE in bf16.
  Q,K carry a scale ALPHA/BETA folded into their PSUM drain so the scores
  PSUM lands directly in Schraudolph-exp units (see below).
- Scores are computed transposed, S^T[t, q] = K_d,t^T . Q_d,q, so the
  softmax mask/shift folds into the exp bias (per-partition), and a
  ones-column appended to V yields softmax denominators as row 64 of the
  context matmul output.
- The softmax exp is engine-split: ACT runs the Exp activation (bf16 out)
  for 12/16 kv tiles; DVE computes the other 4/16 with one tensor_scalar
  op via the Schraudolph bit trick targeted at bf16:
  bits16(exp(z)) ~= (128/ln2) z + 16249.6, and ALPHA*BETA is chosen so the
  scores PSUM already holds (128/ln2) * logit. out_i16 = max(psum, -b) + b
  with b = (128/ln2)(mask-2) + 16249.6; the -2 shift cancels in the
  normalize.
- Context is accumulated TRANSPOSED: ctx[q, d] tiles [128, 65] with the
  just-computed ex tile as the PE stationary operand and V(+ones) as the
  64+1-column moving operand — 65 charged rows per matmul instead of 512
  (the cost model charges moving rows only), halving context PE time.
  The denominator lands in column 64 (per-partition), so normalization is
  a reciprocal plus one per-partition tensor_scalar multiply on DVE, and
  the normalized tile is PE-transposed back to feature-major for o_proj.
- Emission is software-pipelined: after a startup phase that projects the
  first batch's tokens, the remaining projection chunks, V-transposes and
  all o_proj blocks are interleaved between attention g-blocks so the PE
  (the bottleneck engine) never drains. Projection/o_proj/transpose PSUM
  tiles share the two "ops" PSUM bank slots; tiny [1,1] wait-carrier
  matmuls pre-spend slot-reuse semaphore waits so every real PE
  instruction needs at most its one producer wait.
- o_proj uses f32r matmuls (full rate at N=512); output is drained to
  bf16 by DVE and DMA'd at half traffic; the host sums partials in fp32.
"""

import os
import sys

for _p in ("/opt/trn_rl_repo",):
    if _p not in sys.path and os.path.isdir(_p):
        sys.path.insert(0, _p)

import numpy as np
import ml_dtypes

import concourse.bass as bass
import concourse.bacc as bacc
import concourse.tile as tile
from concourse import mybir
from concourse import bass_utils

F32 = mybir.dt.float32
F32R = mybir.dt.float32r
BF16 = mybir.dt.bfloat16
I16 = mybir.dt.int16
AF = mybir.ActivationFunctionType

B = 2
S = 2048
H = 2048
D = 64
N_CORES = 8
QH_PER_CORE = 4          # q-heads per core
QF = QH_PER_CORE * D     # 256 q features per core
TOK = B * S              # 4096
SCALE = 1.0 / np.sqrt(D)  # 0.125
CK = 256                 # projection chunk tokens

# bf16 Schraudolph-exp calibration. scores_psum = ALPHA*BETA*(q.k) = A16*s
# where s = SCALE*(q.k) is the true logit and A16 = 128/ln2 is the bf16
# bits-per-logit slope.
A16 = 128.0 / np.log(2.0)                # 184.664
AB = A16 * SCALE                         # required ALPHA*BETA
ALPHA = float(np.sqrt(AB))               # q drain scale (4.804)
BETA = float(np.sqrt(AB))                # k drain scale
EXP_SHIFT = -2.0                         # exp(s + mask - 2) range centering
C16 = 0.058 * 128.0                      # Schraudolph mean-shift
B16CONST = 127.0 * 128.0 - C16 + 0.5     # +0.5: astype(int16) truncates
ACT_SCALE = float(1.0 / A16)             # ACT-path exp input scale

_CACHE = {}


def _exp_on_dve(t):
    """Which kv tiles the DVE (vs ACT) computes exp for: 4/16 so ACT:DVE
    engine busy stays balanced against their other work."""
    return t % 4 == 3


def _build_program():
    nc = bacc.Bacc("TRN2", target_bir_lowering=False, debug=False)

    hsT = nc.dram_tensor("hsT", [H, TOK], BF16, kind="ExternalInput").ap()
    wqkvT = nc.dram_tensor("wqkvT", [H, 384], BF16, kind="ExternalInput").ap()
    woT = nc.dram_tensor("woT", [QF, H], BF16, kind="ExternalInput").ap()
    bqkv = nc.dram_tensor("bqkv", [128, 3], F32, kind="ExternalInput").ap()
    prep = nc.dram_tensor("prep", [128, B, S // 128, 3], F32, kind="ExternalInput").ap()
    eye = nc.dram_tensor("eye", [128, 128], BF16, kind="ExternalInput").ap()
    out = nc.dram_tensor("out", [B, S, H], BF16, kind="ExternalOutput").ap()

    with tile.TileContext(nc) as tc:
        with tc.tile_pool(name="const", bufs=1) as cp, \
             tc.tile_pool(name="proj_sb", bufs=2) as psb, \
             tc.tile_pool(name="exb_sb", bufs=34) as ebp, \
             tc.tile_pool(name="drain_sb", bufs=3) as dsb, \
             tc.tile_pool(name="osb_sb", bufs=3) as osp, \
             tc.tile_pool(name="ctxT_sb", bufs=3) as csb, \
             tc.tile_pool(name="scores_ps", bufs=2, space="PSUM") as sps, \
             tc.tile_pool(name="ctx_ps", bufs=2, space="PSUM") as xps, \
             tc.tile_pool(name="o_ps", bufs=2, space="PSUM") as ops_pool:
            w_qkv = cp.tile([128, 16, 384], BF16)     # (p, h_tile, feature)
            nc.sync.dma_start(out=w_qkv, in_=wqkvT.rearrange("(t p) f -> p t f", p=128))
            w_o = cp.tile([128, 2, H], BF16)          # (p, f_tile, e)
            nc.sync.dma_start(out=w_o, in_=woT.rearrange("(t p) e -> p t e", p=128))
            bqkv_sb = cp.tile([128, 3], F32)
            nc.sync.dma_start(out=bqkv_sb, in_=bqkv)
            prep_sb = cp.tile([128, B, S // 128, 3], F32)
            nc.sync.dma_start(out=prep_sb, in_=prep)
            eye_sb = cp.tile([128, 128], BF16)
            nc.sync.dma_start(out=eye_sb, in_=eye)

            # Engine wait budgets are tiny (1 sync-wait per instruction for
            # PE/ACT structs). Warm consumer-engine vector clocks on the
            # small const DMAs so real instructions never need extra waits.
            scratch = cp.tile([128, 1], F32)
            nc.scalar.copy(out=scratch, in_=bqkv_sb[:, 0:1])
            nc.scalar.copy(out=scratch, in_=prep_sb[:, 0, 0, 0:1])
            scratch_d = cp.tile([128, 1], F32)
            nc.vector.tensor_copy(out=scratch_d, in_=prep_sb[:, 0, 0, 1:2])

            # Q^T bf16 scaled by ALPHA (2 q-heads per tile), V^T bf16
            # (partitions 64:128), K^T bf16 scaled by BETA replicated in
            # both partition halves (PE base-partition alignment for the
            # per-head scores matmuls).
            qT = [cp.tile([128, TOK], BF16, name=f"qT{i}") for i in range(2)]
            vT = cp.tile([128, TOK], BF16)
            k2 = cp.tile([128, TOK], BF16)
            # V transposed back to [t, d] + ones column, per 128-token tile.
            vones = cp.tile([128, B * 16, 65], BF16)

            hsT_tiled = hsT.rearrange("(t p) n -> p t n", p=128)

            def dummy_mm(ps_tile, reader):
                """[1,1] wait-carrier matmul: first touch of a PSUM slot,
                spending its slot-reuse wait; `reader` must be an SBUF AP
                whose producer clock the PE already knows."""
                nc.tensor.matmul(ps_tile[0:1, 0:1], reader, reader,
                                 start=True, stop=True)

            w_r = w_qkv[:, 0, 0:2].bitcast(F32)[:, 0:1]
            wo_r = w_o[:, 0, 0:2].bitcast(F32)[:, 0:1]
            eye_r = eye_sb[0:64, 0:2].bitcast(F32)[:, 0:1]

            def emit_proj(ck):
                hstage = psb.tile([128, 16, CK], BF16, tag="hstage", name=f"hs_{ck}")
                nc.sync.dma_start(
                    out=hstage, in_=hsT_tiled[:, :, ck * CK:(ck + 1) * CK])
                cols = slice(ck * CK, (ck + 1) * CK)
                for ft in range(3):
                    ps = ops_pool.tile([128, 512], F32, tag="ops", name=f"pj_{ck}_{ft}")
                    # carrier takes the slot-release wait; the first real
                    # matmul then only waits on the hstage DMA.
                    dummy_mm(ps, w_r)
                    for ht in range(16):
                        nc.tensor.matmul(
                            ps[:, 0:CK],
                            w_qkv[:, ht, ft * 128:(ft + 1) * 128],
                            hstage[:, ht, :],
                            start=(ht == 0), stop=(ht == 15),
                        )
                    if ft < 2:
                        nc.scalar.activation(
                            out=qT[ft][:, cols], in_=ps[:, 0:CK],
                            func=AF.Identity, bias=bqkv_sb[:, ft:ft + 1],
                            scale=ALPHA,
                        )
                    else:
                        nc.scalar.activation(
                            out=k2[0:64, cols], in_=ps[0:64, 0:CK],
                            func=AF.Identity, bias=bqkv_sb[0:64, 2:3],
                            scale=BETA,
                        )
                        nc.scalar.activation(
                            out=vT[64:128, cols], in_=ps[64:128, 0:CK],
                            func=AF.Identity, bias=bqkv_sb[64:128, 2:3],
                        )
                # V[t, d] tiles for the two 128-token tiles of this chunk
                # (both share one ops slot at different bf16 column ranges)
                tp = ops_pool.tile([128, 512], F32, tag="ops", name=f"tp_{ck}")
                dummy_mm(tp, eye_r)
                for i, bt in enumerate((2 * ck, 2 * ck + 1)):
                    tpb = tp.bitcast(BF16)[:, 64 * i:64 * i + 64]
                    nc.tensor.transpose(
                        tpb, in_=vT[64:128, bt * 128:(bt + 1) * 128],
                        identity=eye_sb[64:128, 64:128])
                    nc.scalar.copy(out=vones[:, bt, 0:64], in_=tpb)

            def emit_k2_repl(b):
                half = slice(b * S, (b + 1) * S)
                nc.sync.dma_start(out=k2[64:128, half], in_=k2[0:64, half])
                # ones column for this batch's vones tiles (ACT: scale 0)
                nc.scalar.activation(
                    out=vones[:, b * 16:(b + 1) * 16, 64:65],
                    in_=prep_sb[:, b, :, 0],
                    func=AF.Identity, bias=1.0, scale=0.0)
                # spend the k2-replication DMA wait on the PE clock
                dps = sps.tile([128, 1024], F32, tag="scores", name=f"k2d_{b}")
                nc.tensor.matmul(dps[0:1, 0:1],
                                 k2[64:128, b * S:b * S + 2].bitcast(F32)[:, 0:1],
                                 k2[64:128, b * S:b * S + 2].bitcast(F32)[:, 0:1],
                                 start=True, stop=True)

            ctxT_tiles = {}
            exs_map = {}

            def emit_att_sc(b, qh, g):
                q0 = b * S + qh * 1024
                if g == 0:
                    # No slot pre-spend needed: the first writer (the DVE
                    # tensor_copy) and the slot releaser (PE o_proj reads)
                    # pair with the PE-side transpose producer wait.
                    ctxT = [csb.tile([128, 1024], BF16, tag=f"ctxT{ft}",
                                     name=f"ctxT{ft}_{b}_{qh}") for ft in range(2)]
                    ctxT_tiles[(b, qh)] = ctxT
                ctxT = ctxT_tiles[(b, qh)]
                qt = qT[g // 2]
                qp = (g % 2) * 64
                exs = []
                for t in range(16):
                    sc = sps.tile([128, 1024], F32, tag="scores",
                                  name=f"sc_{b}_{qh}_{g}_{t}")
                    for qc in range(2):
                        nc.tensor.matmul(
                            sc[:, qc * 512:(qc + 1) * 512],
                            k2[qp:qp + 64, b * S + t * 128:b * S + (t + 1) * 128],
                            qt[qp:qp + 64, q0 + qc * 512:q0 + (qc + 1) * 512],
                            start=True, stop=True,
                        )
                    ex = ebp.tile([128, 1024], BF16, tag="expT",
                                  name=f"ex_{b}_{qh}_{g}_{t}")
                    exs.append(ex)
                    if _exp_on_dve(t):
                        nc.vector.tensor_scalar(
                            out=ex.bitcast(I16), in0=sc,
                            scalar1=prep_sb[:, b, t, 2:3],
                            scalar2=prep_sb[:, b, t, 1:2],
                            op0=mybir.AluOpType.max,
                            op1=mybir.AluOpType.add,
                        )
                    else:
                        nc.scalar.activation(
                            out=ex, in_=sc, func=AF.Exp,
                            bias=prep_sb[:, b, t, 0:1],
                            scale=ACT_SCALE,
                        )
                exs_map[(b, qh, g)] = exs

            def emit_att_ctx(b, qh, g):
                # transposed context: per 128-token q subtile, accumulate
                # ctx[q, d|den] over all kv tiles with ex as the stationary
                # operand (65 charged rows per matmul), then normalize by
                # the per-partition denominator column and transpose back.
                # Emitted one block late so the PE never head-of-line waits
                # on this block's own exp results.
                ctxT = ctxT_tiles[(b, qh)]
                exs = exs_map.pop((b, qh, g))
                qp = (g % 2) * 64
                for qp2 in range(4):        # pairs of 128-token q subtiles
                    cn = dsb.tile([128, 128], BF16, tag="ctxn",
                                  name=f"cn_{b}_{qh}_{g}_{qp2}")
                    for i in range(2):
                        qs = 2 * qp2 + i
                        cx = xps.tile([128, 65], F32, tag="ctx",
                                      name=f"cx_{b}_{qh}_{g}_{qs}")
                        # wait-carrier: spend the ctx-slot WAR wait (DVE
                        # release) before the real t=0 accumulation start.
                        dummy_mm(cx, wo_r)
                        for t in range(16):
                            nc.tensor.matmul(
                                cx,
                                exs[t][:, qs * 128:(qs + 1) * 128],
                                vones[:, b * 16 + t, :],
                                start=(t == 0), stop=(t == 15),
                            )
                        rc = dsb.tile([128, 1], F32, tag="recip",
                                      name=f"rc_{b}_{qh}_{g}_{qs}")
                        nc.vector.reciprocal(out=rc, in_=cx[:, 64:65])
                        nc.vector.tensor_scalar(
                            out=cn[:, 64 * i:64 * i + 64], in0=cx[:, 0:64],
                            scalar1=rc, scalar2=None,
                            op0=mybir.AluOpType.mult,
                        )
                    # one transpose covers both subtiles: out rows 0:64 are
                    # subtile 2*qp2 features, rows 64:128 the other's
                    tq = ops_pool.tile([128, 512], F32, tag="ops",
                                       name=f"tq_{b}_{qh}_{g}_{qp2}")
                    dummy_mm(tq, eye_r)
                    tqb = tq.bitcast(BF16)[:, 0:128]
                    nc.tensor.transpose(tqb, in_=cn, identity=eye_sb)
                    for i in range(2):
                        qs = 2 * qp2 + i
                        nc.vector.tensor_copy(
                            out=ctxT[g // 2][qp:qp + 64, qs * 128:(qs + 1) * 128],
                            in_=tqb[64 * i:64 * i + 64, :],
                        )

            def emit_oproj(b, qh, qq, tail=False):
                ctxT = ctxT_tiles[(b, qh)]
                osb = osp.tile([128, H], BF16, tag="osb", name=f"osb_{b}_{qh}_{qq}")
                # pre-spend the osb slot-reuse wait (out-DMA done)
                nc.vector.memset(osb[0:1, 0:1], 0.0)
                for ec in range(4):
                    op = ops_pool.tile([128, 512], F32, tag="ops",
                                       name=f"op_{b}_{qh}_{qq}_{ec}")
                    # carrier takes the slot-release wait (mixed ACT/DVE
                    # releasers from interleaved proj drains)
                    dummy_mm(op, wo_r)
                    for ft in range(2):
                        nc.tensor.matmul(
                            op,
                            ctxT[ft][:, qq * 128:(qq + 1) * 128],
                            w_o[:, ft, ec * 512:(ec + 1) * 512],
                            start=(ft == 0), stop=(ft == 1),
                        )
                    if tail and ec % 2 == 1:
                        nc.scalar.copy(
                            out=osb[:, ec * 512:(ec + 1) * 512], in_=op)
                    else:
                        nc.vector.tensor_copy(
                            out=osb[:, ec * 512:(ec + 1) * 512], in_=op)
                nc.sync.dma_start(
                    out=out[b, qh * 1024 + qq * 128:qh * 1024 + (qq + 1) * 128, :],
                    in_=osb,
                )

            # ---- software-pipelined emission schedule ----
            for ck in range(8):
                emit_proj(ck)
            emit_k2_repl(0)

            proj_pending = list(range(8, 16))
            op_pending = []
            att_blocks = [(b, qh, g) for b in range(B) for qh in range(2)
                          for g in range(QH_PER_CORE)]
            prev = None
            for i, (b, qh, g) in enumerate(att_blocks):
                emit_att_sc(b, qh, g)
                if prev is not None:
                    emit_att_ctx(*prev)
                    if prev[2] == QH_PER_CORE - 1:
                        op_pending.extend((prev[0], prev[1], qq) for qq in range(8))
                prev = (b, qh, g)
                if i == 7:
                    # b=1 attention needs all projections + its K replica
                    while proj_pending:
                        emit_proj(proj_pending.pop(0))
                    emit_k2_repl(1)
                    continue
                if proj_pending:
                    emit_proj(proj_pending.pop(0))
                    n_op = 1
                else:
                    n_op = 3
                for _ in range(min(n_op, len(op_pending))):
                    emit_oproj(*op_pending.pop(0))
            emit_att_ctx(*prev)
            op_pending.extend((prev[0], prev[1], qq) for qq in range(8))
            while op_pending:
                emit_oproj(*op_pending.pop(0), tail=True)
    nc.compile()
    return nc


def kernel(hidden_states, attention_mask, Wq, bq, Wk, bk, Wv, bv, Wo, bo):
    hidden_states = np.asarray(hidden_states, dtype=np.float32)
    attention_mask = np.asarray(attention_mask, dtype=np.float32)
    Wq = np.asarray(Wq, dtype=np.float32)
    Wk = np.asarray(Wk, dtype=np.float32)
    Wv = np.asarray(Wv, dtype=np.float32)
    Wo = np.asarray(Wo, dtype=np.float32)

    if "nc" not in _CACHE:
        _CACHE["nc"] = _build_program()
    nc = _CACHE["nc"]

    hsT = np.ascontiguousarray(
        hidden_states.reshape(TOK, H).T).astype(ml_dtypes.bfloat16)  # [H, B*S]
    maskp = np.ascontiguousarray(
        attention_mask.reshape(B, S // 128, 128).transpose(2, 0, 1))  # [128, B, 16]
    sb16 = A16 * (maskp + EXP_SHIFT) + B16CONST
    prep = np.stack([maskp + EXP_SHIFT, sb16, -sb16], axis=-1).astype(np.float32)
    prep = np.ascontiguousarray(prep)                     # [128, B, 16, 3]
    eye = np.eye(128, dtype=np.float32).astype(ml_dtypes.bfloat16)

    in_maps = []
    for c in range(N_CORES):
        wq = Wq[QF * c:QF * (c + 1)]          # [256, H]
        wk = Wk[D * c:D * (c + 1)]            # [64, H]
        wv = Wv[D * c:D * (c + 1)]            # [64, H]
        wqkvT = np.ascontiguousarray(
            np.concatenate([wq, wk, wv], axis=0).T).astype(ml_dtypes.bfloat16)
        woT = np.ascontiguousarray(
            Wo[:, QF * c:QF * (c + 1)].T).astype(ml_dtypes.bfloat16)          # [256, H]
        bq_c = bq[QF * c:QF * (c + 1)] * ALPHA
        bk_c = bk[D * c:D * (c + 1)] * BETA
        bv_c = bv[D * c:D * (c + 1)]
        bqkv_c = np.ascontiguousarray(
            np.concatenate([bq_c, bk_c, bv_c]).astype(np.float32)
            .reshape(3, 128).T)               # [128, 3]
        in_maps.append({
            "hsT": hsT, "wqkvT": wqkvT, "woT": woT,
            "bqkv": bqkv_c, "prep": prep, "eye": eye,
        })

    _CACHE["last_in_maps"] = in_maps
    res = bass_utils.run_bass_kernel_spmd(nc, in_maps, core_ids=list(range(N_CORES)))
    acc = np.zeros((B, S, H), dtype=np.float32)
    for c in range(N_CORES):
        acc += np.asarray(res.results[c]["out"], dtype=np.float32)
    acc += np.asarray(bo, dtype=np.float32)[None, None, :]
    return acc


# revision 35
# speedup vs baseline: 1.4064x; 1.0182x over previous
"""GQA attention kernel for Trainium2, sharded over 8 NeuronCores.

Sharding: tensor-parallel over heads. Core c owns kv-head c and q-heads
4c..4c+3 (rows 256c:256c+256 of Wq, rows 64c:64c+64 of Wk/Wv) and columns
256c:256c+256 of Wo. Each core computes a full-shape partial of the output
(o_proj column-parallel); the host sums the 8 partials (the all-reduce)
and adds bo.

Per-core kernel layout choices:
- hidden_states is passed transposed [H, B*S] in bf16 so QKV projections
  contract over the partition dim with one big contiguous DMA per 256-token
  chunk (descriptor-generation time is per-DMA).
- Q,K,V are produced transposed ([feature, token]) by the PE in bf16.
  Q,K carry a scale ALPHA/BETA folded into their PSUM drain so the scores
  PSUM lands directly in Schraudolph-exp units (see below).
- Scores are computed transposed, S^T[t, q] = K_d,t^T . Q_d,q, so the
  softmax mask/shift folds into the exp bias (per-partition), and a
  ones-column appended to V yields softmax denominators as row 64 of the
  context matmul output.
- The softmax exp is engine-split: ACT runs the Exp activation (bf16 out)
  for 12/16 kv tiles; DVE computes the other 4/16 with one tensor_scalar
  op via the Schraudolph bit trick targeted at bf16:
  bits16(exp(z)) ~= (128/ln2) z + 16249.6, and ALPHA*BETA is chosen so the
  scores PSUM already holds (128/ln2) * logit. out_i16 = max(psum, -b) + b
  with b = (128/ln2)(mask-2) + 16249.6; the -2 shift cancels in the
  normalize.
- Context is accumulated TRANSPOSED: ctx[q, d] tiles [128, 65] with the
  just-computed ex tile as the PE stationary operand and V(+ones) as the
  64+1-column moving operand — 65 charged rows per matmul instead of 512
  (the cost model charges moving rows only), halving context PE time.
  The denominator lands in column 64 (per-partition), so normalization is
  a reciprocal plus one per-partition tensor_scalar multiply on DVE, and
  the normalized tile is PE-transposed back to feature-major for o_proj.
- Emission is software-pipelined: after a startup phase that projects the
  first batch's tokens, the remaining projection chunks, V-transposes and
  all o_proj blocks are interleaved between attention g-blocks so the PE
  (the bottleneck engine) never drains. Projection/o_proj/transpose PSUM
  tiles share the two "ops" PSUM bank slots; tiny [1,1] wait-carrier
  matmuls pre-spend slot-reuse semaphore waits so every real PE
  instruction needs at most its one producer wait.
- o_proj uses f32r matmuls (full rate at N=512); output is drained to
  bf16 by DVE and DMA'd at half traffic; the host sums partials in fp32.
"""

import os
import sys

for _p in ("/opt/trn_rl_repo",):
    if _p not in sys.path and os.path.isdir(_p):
        sys.path.insert(0, _p)

import numpy as np
import ml_dtypes

import concourse.bass as bass
import concourse.bacc as bacc
import concourse.tile as tile
from concourse import mybir
from concourse import bass_utils

F32 = mybir.dt.float32
F32R = mybir.dt.float32r
BF16 = mybir.dt.bfloat16
I16 = mybir.dt.int16
AF = mybir.ActivationFunctionType

B = 2
S = 2048
H = 2048
D = 64
N_CORES = 8
QH_PER_CORE = 4          # q-heads per core
QF = QH_PER_CORE * D     # 256 q features per core
TOK = B * S              # 4096
SCALE = 1.0 / np.sqrt(D)  # 0.125
CK = 256                 # projection chunk tokens

# bf16 Schraudolph-exp calibration. scores_psum = ALPHA*BETA*(q.k) = A16*s
# where s = SCALE*(q.k) is the true logit and A16 = 128/ln2 is the bf16
# bits-per-logit slope.
A16 = 128.0 / np.log(2.0)                # 184.664
AB = A16 * SCALE                         # required ALPHA*BETA
ALPHA = float(np.sqrt(AB))               # q drain scale (4.804)
BETA = float(np.sqrt(AB))                # k drain scale
EXP_SHIFT = -2.0                         # exp(s + mask - 2) range centering
C16 = 0.058 * 128.0                      # Schraudolph mean-shift
B16CONST = 127.0 * 128.0 - C16 + 0.5     # +0.5: astype(int16) truncates
ACT_SCALE = float(1.0 / A16)             # ACT-path exp input scale

_CACHE = {}


def _exp_on_dve(t):
    """Which kv tiles the DVE (vs ACT) computes exp for: 4/16 so ACT:DVE
    engine busy stays balanced against their other work."""
    return t % 4 == 3


def _build_program():
    nc = bacc.Bacc("TRN2", target_bir_lowering=False, debug=False)

    hsT = nc.dram_tensor("hsT", [H, TOK], BF16, kind="ExternalInput").ap()
    wqkvT = nc.dram_tensor("wqkvT", [H, 384], BF16, kind="ExternalInput").ap()
    woT = nc.dram_tensor("woT", [QF, H], BF16, kind="ExternalInput").ap()
    bqkv = nc.dram_tensor("bqkv", [128, 3], F32, kind="ExternalInput").ap()
    prep = nc.dram_tensor("prep", [128, B, S // 128, 3], F32, kind="ExternalInput").ap()
    eye = nc.dram_tensor("eye", [128, 128], BF16, kind="ExternalInput").ap()
    out = nc.dram_tensor("out", [B, S, H], BF16, kind="ExternalOutput").ap()

    with tile.TileContext(nc) as tc:
        with tc.tile_pool(name="const", bufs=1) as cp, \
             tc.tile_pool(name="proj_sb", bufs=3) as psb, \
             tc.tile_pool(name="exb_sb", bufs=34) as ebp, \
             tc.tile_pool(name="drain_sb", bufs=3) as dsb, \
             tc.tile_pool(name="osb_sb", bufs=3) as osp, \
             tc.tile_pool(name="ctxT_sb", bufs=3) as csb, \
             tc.tile_pool(name="scores_ps", bufs=2, space="PSUM") as sps, \
             tc.tile_pool(name="ctx_ps", bufs=2, space="PSUM") as xps, \
             tc.tile_pool(name="o_ps", bufs=2, space="PSUM") as ops_pool:
            w_qkv = cp.tile([128, 16, 384], BF16)     # (p, h_tile, feature)
            nc.sync.dma_start(out=w_qkv, in_=wqkvT.rearrange("(t p) f -> p t f", p=128))
            w_o = cp.tile([128, 2, H], BF16)          # (p, f_tile, e)
            nc.sync.dma_start(out=w_o, in_=woT.rearrange("(t p) e -> p t e", p=128))
            bqkv_sb = cp.tile([128, 3], F32)
            nc.sync.dma_start(out=bqkv_sb, in_=bqkv)
            prep_sb = cp.tile([128, B, S // 128, 3], F32)
            nc.sync.dma_start(out=prep_sb, in_=prep)
            eye_sb = cp.tile([128, 128], BF16)
            nc.sync.dma_start(out=eye_sb, in_=eye)

            # Engine wait budgets are tiny (1 sync-wait per instruction for
            # PE/ACT structs). Warm consumer-engine vector clocks on the
            # small const DMAs so real instructions never need extra waits.
            scratch = cp.tile([128, 1], F32)
            nc.scalar.copy(out=scratch, in_=bqkv_sb[:, 0:1])
            nc.scalar.copy(out=scratch, in_=prep_sb[:, 0, 0, 0:1])
            scratch_d = cp.tile([128, 1], F32)
            nc.vector.tensor_copy(out=scratch_d, in_=prep_sb[:, 0, 0, 1:2])

            # Q^T bf16 scaled by ALPHA (2 q-heads per tile), V^T bf16
            # (partitions 64:128), K^T bf16 scaled by BETA replicated in
            # both partition halves (PE base-partition alignment for the
            # per-head scores matmuls).
            qT = [cp.tile([128, TOK], BF16, name=f"qT{i}") for i in range(2)]
            vT = cp.tile([128, TOK], BF16)
            k2 = cp.tile([128, TOK], BF16)
            # V transposed back to [t, d] + ones column, per 128-token tile.
            vones = cp.tile([128, B * 16, 65], BF16)

            hsT_tiled = hsT.rearrange("(t p) n -> p t n", p=128)

            def dummy_mm(ps_tile, reader):
                """[1,1] wait-carrier matmul: first touch of a PSUM slot,
                spending its slot-reuse wait; `reader` must be an SBUF AP
                whose producer clock the PE already knows."""
                nc.tensor.matmul(ps_tile[0:1, 0:1], reader, reader,
                                 start=True, stop=True)

            w_r = w_qkv[:, 0, 0:2].bitcast(F32)[:, 0:1]
            wo_r = w_o[:, 0, 0:2].bitcast(F32)[:, 0:1]
            eye_r = eye_sb[0:64, 0:2].bitcast(F32)[:, 0:1]

            def emit_proj(ck, startup=False):
                hstage = psb.tile([128, 16, CK], BF16, tag="hstage", name=f"hs_{ck}")
                nc.sync.dma_start(
                    out=hstage, in_=hsT_tiled[:, :, ck * CK:(ck + 1) * CK])
                cols = slice(ck * CK, (ck + 1) * CK)
                for ft in range(3):
                    # during startup the attention ctx PSUM slots are idle;
                    # alternating pools doubles the slot-recycle distance
                    pool = xps if startup and ft == 1 else ops_pool
                    ps = pool.tile([128, 512], F32, tag="ctx" if pool is xps else "ops",
                                   name=f"pj_{ck}_{ft}")
                    # carrier takes the slot-release wait; the first real
                    # matmul then only waits on the hstage DMA.
                    dummy_mm(ps, w_r)
                    for ht in range(16):
                        nc.tensor.matmul(
                            ps[:, 0:CK],
                            w_qkv[:, ht, ft * 128:(ft + 1) * 128],
                            hstage[:, ht, :],
                            start=(ht == 0), stop=(ht == 15),
                        )
                    if ft < 2:
                        nc.scalar.activation(
                            out=qT[ft][:, cols], in_=ps[:, 0:CK],
                            func=AF.Identity, bias=bqkv_sb[:, ft:ft + 1],
                            scale=ALPHA,
                        )
                    else:
                        nc.scalar.activation(
                            out=k2[0:64, cols], in_=ps[0:64, 0:CK],
                            func=AF.Identity, bias=bqkv_sb[0:64, 2:3],
                            scale=BETA,
                        )
                        nc.scalar.activation(
                            out=vT[64:128, cols], in_=ps[64:128, 0:CK],
                            func=AF.Identity, bias=bqkv_sb[64:128, 2:3],
                        )
                # V[t, d] tiles for the two 128-token tiles of this chunk
                # (both share one ops slot at different bf16 column ranges)
                pool = xps if startup else ops_pool
                tp = pool.tile([128, 512], F32, tag="ctx" if startup else "ops",
                               name=f"tp_{ck}")
                dummy_mm(tp, eye_r)
                for i, bt in enumerate((2 * ck, 2 * ck + 1)):
                    tpb = tp.bitcast(BF16)[:, 64 * i:64 * i + 64]
                    nc.tensor.transpose(
                        tpb, in_=vT[64:128, bt * 128:(bt + 1) * 128],
                        identity=eye_sb[64:128, 64:128])
                    nc.scalar.copy(out=vones[:, bt, 0:64], in_=tpb)

            def emit_k2_repl(b):
                half = slice(b * S, (b + 1) * S)
                nc.sync.dma_start(out=k2[64:128, half], in_=k2[0:64, half])
                # ones column for this batch's vones tiles (ACT: scale 0)
                nc.scalar.activation(
                    out=vones[:, b * 16:(b + 1) * 16, 64:65],
                    in_=prep_sb[:, b, :, 0],
                    func=AF.Identity, bias=1.0, scale=0.0)
                # spend the k2-replication DMA wait on the PE clock
                dps = sps.tile([128, 1024], F32, tag="scores", name=f"k2d_{b}")
                nc.tensor.matmul(dps[0:1, 0:1],
                                 k2[64:128, b * S:b * S + 2].bitcast(F32)[:, 0:1],
                                 k2[64:128, b * S:b * S + 2].bitcast(F32)[:, 0:1],
                                 start=True, stop=True)

            ctxT_tiles = {}
            exs_map = {}

            def emit_att_sc(b, qh, g):
                q0 = b * S + qh * 1024
                if g == 0:
                    # No slot pre-spend needed: the first writer (the DVE
                    # tensor_copy) and the slot releaser (PE o_proj reads)
                    # pair with the PE-side transpose producer wait.
                    ctxT = [csb.tile([128, 1024], BF16, tag=f"ctxT{ft}",
                                     name=f"ctxT{ft}_{b}_{qh}") for ft in range(2)]
                    ctxT_tiles[(b, qh)] = ctxT
                ctxT = ctxT_tiles[(b, qh)]
                qt = qT[g // 2]
                qp = (g % 2) * 64
                exs = []
                for t in range(16):
                    sc = sps.tile([128, 1024], F32, tag="scores",
                                  name=f"sc_{b}_{qh}_{g}_{t}")
                    for qc in range(2):
                        nc.tensor.matmul(
                            sc[:, qc * 512:(qc + 1) * 512],
                            k2[qp:qp + 64, b * S + t * 128:b * S + (t + 1) * 128],
                            qt[qp:qp + 64, q0 + qc * 512:q0 + (qc + 1) * 512],
                            start=True, stop=True,
                        )
                    ex = ebp.tile([128, 1024], BF16, tag="expT",
                                  name=f"ex_{b}_{qh}_{g}_{t}")
                    exs.append(ex)
                    if _exp_on_dve(t):
                        nc.vector.tensor_scalar(
                            out=ex.bitcast(I16), in0=sc,
                            scalar1=prep_sb[:, b, t, 2:3],
                            scalar2=prep_sb[:, b, t, 1:2],
                            op0=mybir.AluOpType.max,
                            op1=mybir.AluOpType.add,
                        )
                    else:
                        nc.scalar.activation(
                            out=ex, in_=sc, func=AF.Exp,
                            bias=prep_sb[:, b, t, 0:1],
                            scale=ACT_SCALE,
                        )
                exs_map[(b, qh, g)] = exs

            def emit_att_ctx(b, qh, g):
                # transposed context: per 128-token q subtile, accumulate
                # ctx[q, d|den] over all kv tiles with ex as the stationary
                # operand (65 charged rows per matmul), then normalize by
                # the per-partition denominator column and transpose back.
                # Emitted one block late so the PE never head-of-line waits
                # on this block's own exp results.
                ctxT = ctxT_tiles[(b, qh)]
                exs = exs_map.pop((b, qh, g))
                qp = (g % 2) * 64
                for qp2 in range(4):        # pairs of 128-token q subtiles
                    cn = dsb.tile([128, 128], BF16, tag="ctxn",
                                  name=f"cn_{b}_{qh}_{g}_{qp2}")
                    for i in range(2):
                        qs = 2 * qp2 + i
                        cx = xps.tile([128, 65], F32, tag="ctx",
                                      name=f"cx_{b}_{qh}_{g}_{qs}")
                        # wait-carrier: spend the ctx-slot WAR wait (DVE
                        # release) before the real t=0 accumulation start.
                        dummy_mm(cx, wo_r)
                        for t in range(16):
                            nc.tensor.matmul(
                                cx,
                                exs[t][:, qs * 128:(qs + 1) * 128],
                                vones[:, b * 16 + t, :],
                                start=(t == 0), stop=(t == 15),
                            )
                        rc = dsb.tile([128, 1], F32, tag="recip",
                                      name=f"rc_{b}_{qh}_{g}_{qs}")
                        nc.vector.reciprocal(out=rc, in_=cx[:, 64:65])
                        nc.vector.tensor_scalar(
                            out=cn[:, 64 * i:64 * i + 64], in0=cx[:, 0:64],
                            scalar1=rc, scalar2=None,
                            op0=mybir.AluOpType.mult,
                        )
                    # one transpose covers both subtiles: out rows 0:64 are
                    # subtile 2*qp2 features, rows 64:128 the other's
                    tq = ops_pool.tile([128, 512], F32, tag="ops",
                                       name=f"tq_{b}_{qh}_{g}_{qp2}")
                    dummy_mm(tq, eye_r)
                    tqb = tq.bitcast(BF16)[:, 0:128]
                    nc.tensor.transpose(tqb, in_=cn, identity=eye_sb)
                    for i in range(2):
                        qs = 2 * qp2 + i
                        nc.vector.tensor_copy(
                            out=ctxT[g // 2][qp:qp + 64, qs * 128:(qs + 1) * 128],
                            in_=tqb[64 * i:64 * i + 64, :],
                        )

            def emit_oproj(b, qh, qq, tail=False):
                ctxT = ctxT_tiles[(b, qh)]
                osb = osp.tile([128, H], BF16, tag="osb", name=f"osb_{b}_{qh}_{qq}")
                # pre-spend the osb slot-reuse wait (out-DMA done)
                nc.vector.memset(osb[0:1, 0:1], 0.0)
                for ec in range(4):
                    op = ops_pool.tile([128, 512], F32, tag="ops",
                                       name=f"op_{b}_{qh}_{qq}_{ec}")
                    # carrier takes the slot-release wait (mixed ACT/DVE
                    # releasers from interleaved proj drains)
                    dummy_mm(op, wo_r)
                    for ft in range(2):
                        nc.tensor.matmul(
                            op,
                            ctxT[ft][:, qq * 128:(qq + 1) * 128],
                            w_o[:, ft, ec * 512:(ec + 1) * 512],
                            start=(ft == 0), stop=(ft == 1),
                        )
                    if tail and ec % 2 == 1:
                        nc.scalar.copy(
                            out=osb[:, ec * 512:(ec + 1) * 512], in_=op)
                    else:
                        nc.vector.tensor_copy(
                            out=osb[:, ec * 512:(ec + 1) * 512], in_=op)
                nc.sync.dma_start(
                    out=out[b, qh * 1024 + qq * 128:qh * 1024 + (qq + 1) * 128, :],
                    in_=osb,
                )

            # ---- software-pipelined emission schedule ----
            for ck in range(8):
                emit_proj(ck, startup=True)
            emit_k2_repl(0)

            proj_pending = list(range(8, 16))
            op_pending = []
            att_blocks = [(b, qh, g) for b in range(B) for qh in range(2)
                          for g in range(QH_PER_CORE)]
            prev = None
            for i, (b, qh, g) in enumerate(att_blocks):
                emit_att_sc(b, qh, g)
                if prev is not None:
                    emit_att_ctx(*prev)
                    if prev[2] == QH_PER_CORE - 1:
                        op_pending.extend((prev[0], prev[1], qq) for qq in range(8))
                prev = (b, qh, g)
                if i == 7:
                    # b=1 attention needs all projections + its K replica
                    while proj_pending:
                        emit_proj(proj_pending.pop(0))
                    emit_k2_repl(1)
                    continue
                if proj_pending:
                    emit_proj(proj_pending.pop(0))
                    n_op = 1
                else:
                    n_op = 3
                for _ in range(min(n_op, len(op_pending))):
                    emit_oproj(*op_pending.pop(0))
            emit_att_ctx(*prev)
            op_pending.extend((prev[0], prev[1], qq) for qq in range(8))
            while op_pending:
                emit_oproj(*op_pending.pop(0), tail=True)
    nc.compile()
    return nc


def kernel(hidden_states, attention_mask, Wq, bq, Wk, bk, Wv, bv, Wo, bo):
    hidden_states = np.asarray(hidden_states, dtype=np.float32)
    attention_mask = np.asarray(attention_mask, dtype=np.float32)
    Wq = np.asarray(Wq, dtype=np.float32)
    Wk = np.asarray(Wk, dtype=np.float32)
    Wv = np.asarray(Wv, dtype=np.float32)
    Wo = np.asarray(Wo, dtype=np.float32)

    if "nc" not in _CACHE:
        _CACHE["nc"] = _build_program()
    nc = _CACHE["nc"]

    hsT = np.ascontiguousarray(
        hidden_states.reshape(TOK, H).T).astype(ml_dtypes.bfloat16)  # [H, B*S]
    maskp = np.ascontiguousarray(
        attention_mask.reshape(B, S // 128, 128).transpose(2, 0, 1))  # [128, B, 16]
    sb16 = A16 * (maskp + EXP_SHIFT) + B16CONST
    prep = np.stack([maskp + EXP_SHIFT, sb16, -sb16], axis=-1).astype(np.float32)
    prep = np.ascontiguousarray(prep)                     # [128, B, 16, 3]
    eye = np.eye(128, dtype=np.float32).astype(ml_dtypes.bfloat16)

    in_maps = []
    for c in range(N_CORES):
        wq = Wq[QF * c:QF * (c + 1)]          # [256, H]
        wk = Wk[D * c:D * (c + 1)]            # [64, H]
        wv = Wv[D * c:D * (c + 1)]            # [64, H]
        wqkvT = np.ascontiguousarray(
            np.concatenate([wq, wk, wv], axis=0).T).astype(ml_dtypes.bfloat16)
        woT = np.ascontiguousarray(
            Wo[:, QF * c:QF * (c + 1)].T).astype(ml_dtypes.bfloat16)          # [256, H]
        bq_c = bq[QF * c:QF * (c + 1)] * ALPHA
        bk_c = bk[D * c:D * (c + 1)] * BETA
        bv_c = bv[D * c:D * (c + 1)]
        bqkv_c = np.ascontiguousarray(
            np.concatenate([bq_c, bk_c, bv_c]).astype(np.float32)
            .reshape(3, 128).T)               # [128, 3]
        in_maps.append({
            "hsT": hsT, "wqkvT": wqkvT, "woT": woT,
            "bqkv": bqkv_c, "prep": prep, "eye": eye,
        })

    _CACHE["last_in_maps"] = in_maps
    res = bass_utils.run_bass_kernel_spmd(nc, in_maps, core_ids=list(range(N_CORES)))
    acc = np.zeros((B, S, H), dtype=np.float32)
    for c in range(N_CORES):
        acc += np.asarray(res.results[c]["out"], dtype=np.float32)
    acc += np.asarray(bo, dtype=np.float32)[None, None, :]
    return acc


# revision 36
# speedup vs baseline: 1.4511x; 1.0317x over previous
"""GQA attention kernel for Trainium2, sharded over 8 NeuronCores.

Sharding: tensor-parallel over heads. Core c owns kv-head c and q-heads
4c..4c+3 (rows 256c:256c+256 of Wq, rows 64c:64c+64 of Wk/Wv) and columns
256c:256c+256 of Wo. Each core computes a full-shape partial of the output
(o_proj column-parallel); the host sums the 8 partials (the all-reduce)
and adds bo.

Per-core kernel layout choices:
- hidden_states is passed transposed [H, B*S] in bf16 so QKV projections
  contract over the partition dim with one big contiguous DMA per 256-token
  chunk (descriptor-generation time is per-DMA).
- Q,K,V are produced transposed ([feature, token]) by the PE in bf16.
  Q,K carry a scale ALPHA/BETA folded into their PSUM drain so the scores
  PSUM lands directly in Schraudolph-exp units (see below).
- Scores are computed transposed, S^T[t, q] = K_d,t^T . Q_d,q, so the
  softmax mask/shift folds into the exp bias (per-partition), and a
  ones-column appended to V yields softmax denominators as row 64 of the
  context matmul output.
- The softmax exp is engine-split: ACT runs the Exp activation (bf16 out)
  for 12/16 kv tiles; DVE computes the other 4/16 with one tensor_scalar
  op via the Schraudolph bit trick targeted at bf16:
  bits16(exp(z)) ~= (128/ln2) z + 16249.6, and ALPHA*BETA is chosen so the
  scores PSUM already holds (128/ln2) * logit. out_i16 = max(psum, -b) + b
  with b = (128/ln2)(mask-2) + 16249.6; the -2 shift cancels in the
  normalize.
- Context is accumulated TRANSPOSED: ctx[q, d] tiles [128, 65] with the
  just-computed ex tile as the PE stationary operand and V(+ones) as the
  64+1-column moving operand — 65 charged rows per matmul instead of 512
  (the cost model charges moving rows only), halving context PE time.
  The denominator lands in column 64 (per-partition), so normalization is
  a reciprocal plus one per-partition tensor_scalar multiply on DVE, and
  the normalized tile is PE-transposed back to feature-major for o_proj.
- Emission is software-pipelined: after a startup phase that projects the
  first batch's tokens, the remaining projection chunks, V-transposes and
  all o_proj blocks are interleaved between attention g-blocks so the PE
  (the bottleneck engine) never drains. Projection/o_proj/transpose PSUM
  tiles share the two "ops" PSUM bank slots; tiny [1,1] wait-carrier
  matmuls pre-spend slot-reuse semaphore waits so every real PE
  instruction needs at most its one producer wait.
- o_proj uses f32r matmuls (full rate at N=512); output is drained to
  bf16 by DVE and DMA'd at half traffic; the host sums partials in fp32.
"""

import os
import sys

for _p in ("/opt/trn_rl_repo",):
    if _p not in sys.path and os.path.isdir(_p):
        sys.path.insert(0, _p)

import numpy as np
import ml_dtypes

import concourse.bass as bass
import concourse.bacc as bacc
import concourse.tile as tile
from concourse import mybir
from concourse import bass_utils

F32 = mybir.dt.float32
F32R = mybir.dt.float32r
BF16 = mybir.dt.bfloat16
I16 = mybir.dt.int16
AF = mybir.ActivationFunctionType

B = 2
S = 2048
H = 2048
D = 64
N_CORES = 8
QH_PER_CORE = 4          # q-heads per core
QF = QH_PER_CORE * D     # 256 q features per core
TOK = B * S              # 4096
SCALE = 1.0 / np.sqrt(D)  # 0.125
CK = 256                 # projection chunk tokens

# bf16 Schraudolph-exp calibration. scores_psum = ALPHA*BETA*(q.k) = A16*s
# where s = SCALE*(q.k) is the true logit and A16 = 128/ln2 is the bf16
# bits-per-logit slope.
A16 = 128.0 / np.log(2.0)                # 184.664
AB = A16 * SCALE                         # required ALPHA*BETA
ALPHA = float(np.sqrt(AB))               # q drain scale (4.804)
BETA = float(np.sqrt(AB))                # k drain scale
EXP_SHIFT = -2.0                         # exp(s + mask - 2) range centering
C16 = 0.058 * 128.0                      # Schraudolph mean-shift
B16CONST = 127.0 * 128.0 - C16 + 0.5     # +0.5: astype(int16) truncates
ACT_SCALE = float(1.0 / A16)             # ACT-path exp input scale

_CACHE = {}


def _exp_on_dve(t):
    """Which kv tiles the DVE (vs ACT) computes exp for: 4/16 so ACT:DVE
    engine busy stays balanced against their other work."""
    return t % 4 == 3


def _build_program():
    nc = bacc.Bacc("TRN2", target_bir_lowering=False, debug=False)

    hsT = nc.dram_tensor("hsT", [H, TOK], BF16, kind="ExternalInput").ap()
    wqkvT = nc.dram_tensor("wqkvT", [H, 384], BF16, kind="ExternalInput").ap()
    woT = nc.dram_tensor("woT", [QF, H], BF16, kind="ExternalInput").ap()
    bqkv = nc.dram_tensor("bqkv", [128, 3], F32, kind="ExternalInput").ap()
    prep = nc.dram_tensor("prep", [128, B, S // 128, 3], F32, kind="ExternalInput").ap()
    eye = nc.dram_tensor("eye", [128, 128], BF16, kind="ExternalInput").ap()
    out = nc.dram_tensor("out", [B, S, H], BF16, kind="ExternalOutput").ap()

    with tile.TileContext(nc) as tc:
        with tc.tile_pool(name="const", bufs=1) as cp, \
             tc.tile_pool(name="proj_sb", bufs=3) as psb, \
             tc.tile_pool(name="exb_sb", bufs=34) as ebp, \
             tc.tile_pool(name="drain_sb", bufs=3) as dsb, \
             tc.tile_pool(name="osb_sb", bufs=3) as osp, \
             tc.tile_pool(name="ctxT_sb", bufs=3) as csb, \
             tc.tile_pool(name="scores_ps", bufs=2, space="PSUM") as sps, \
             tc.tile_pool(name="ctx_ps", bufs=2, space="PSUM") as xps, \
             tc.tile_pool(name="o_ps", bufs=2, space="PSUM") as ops_pool:
            w_qkv = cp.tile([128, 16, 384], BF16)     # (p, h_tile, feature)
            nc.sync.dma_start(out=w_qkv, in_=wqkvT.rearrange("(t p) f -> p t f", p=128))
            bqkv_sb = cp.tile([128, 3], F32)
            nc.sync.dma_start(out=bqkv_sb, in_=bqkv)
            eye_sb = cp.tile([128, 128], BF16)
            nc.sync.dma_start(out=eye_sb, in_=eye)
            # w_o and prep are first consumed tens of microseconds in; issue
            # their DMAs behind the first projection chunk's token loads.
            w_o = cp.tile([128, 2, H], BF16)          # (p, f_tile, e)
            prep_sb = cp.tile([128, B, S // 128, 3], F32)

            def emit_late_consts():
                nc.sync.dma_start(out=w_o, in_=woT.rearrange("(t p) e -> p t e", p=128))
                nc.sync.dma_start(out=prep_sb, in_=prep)

            # Engine wait budgets are tiny (1 sync-wait per instruction for
            # PE/ACT structs). Warm consumer-engine vector clocks on the
            # small const DMAs so real instructions never need extra waits.
            scratch = cp.tile([128, 1], F32)
            nc.scalar.copy(out=scratch, in_=bqkv_sb[:, 0:1])
            nc.scalar.copy(out=scratch, in_=prep_sb[:, 0, 0, 0:1])
            scratch_d = cp.tile([128, 1], F32)
            nc.vector.tensor_copy(out=scratch_d, in_=prep_sb[:, 0, 0, 1:2])

            # Q^T bf16 scaled by ALPHA (2 q-heads per tile), V^T bf16
            # (partitions 64:128), K^T bf16 scaled by BETA replicated in
            # both partition halves (PE base-partition alignment for the
            # per-head scores matmuls).
            qT = [cp.tile([128, TOK], BF16, name=f"qT{i}") for i in range(2)]
            vT = cp.tile([128, TOK], BF16)
            k2 = cp.tile([128, TOK], BF16)
            # V transposed back to [t, d] + ones column, per 128-token tile.
            vones = cp.tile([128, B * 16, 65], BF16)

            hsT_tiled = hsT.rearrange("(t p) n -> p t n", p=128)

            def dummy_mm(ps_tile, reader):
                """[1,1] wait-carrier matmul: first touch of a PSUM slot,
                spending its slot-reuse wait; `reader` must be an SBUF AP
                whose producer clock the PE already knows."""
                nc.tensor.matmul(ps_tile[0:1, 0:1], reader, reader,
                                 start=True, stop=True)

            w_r = w_qkv[:, 0, 0:2].bitcast(F32)[:, 0:1]
            wo_r = w_o[:, 0, 0:2].bitcast(F32)[:, 0:1]
            eye_r = eye_sb[0:64, 0:2].bitcast(F32)[:, 0:1]

            def emit_proj(ck, startup=False):
                hstage = psb.tile([128, 16, CK], BF16, tag="hstage", name=f"hs_{ck}")
                nc.sync.dma_start(
                    out=hstage, in_=hsT_tiled[:, :, ck * CK:(ck + 1) * CK])
                cols = slice(ck * CK, (ck + 1) * CK)
                for ft in range(3):
                    # during startup the attention ctx PSUM slots are idle;
                    # alternating pools doubles the slot-recycle distance
                    pool = xps if startup and ft == 1 else ops_pool
                    ps = pool.tile([128, 512], F32, tag="ctx" if pool is xps else "ops",
                                   name=f"pj_{ck}_{ft}")
                    # carrier takes the slot-release wait; the first real
                    # matmul then only waits on the hstage DMA.
                    dummy_mm(ps, w_r)
                    for ht in range(16):
                        nc.tensor.matmul(
                            ps[:, 0:CK],
                            w_qkv[:, ht, ft * 128:(ft + 1) * 128],
                            hstage[:, ht, :],
                            start=(ht == 0), stop=(ht == 15),
                        )
                    if ft < 2:
                        nc.scalar.activation(
                            out=qT[ft][:, cols], in_=ps[:, 0:CK],
                            func=AF.Identity, bias=bqkv_sb[:, ft:ft + 1],
                            scale=ALPHA,
                        )
                    else:
                        nc.scalar.activation(
                            out=k2[0:64, cols], in_=ps[0:64, 0:CK],
                            func=AF.Identity, bias=bqkv_sb[0:64, 2:3],
                            scale=BETA,
                        )
                        nc.scalar.activation(
                            out=vT[64:128, cols], in_=ps[64:128, 0:CK],
                            func=AF.Identity, bias=bqkv_sb[64:128, 2:3],
                        )
                # V[t, d] tiles for the two 128-token tiles of this chunk
                # (both share one ops slot at different bf16 column ranges)
                pool = xps if startup else ops_pool
                tp = pool.tile([128, 512], F32, tag="ctx" if startup else "ops",
                               name=f"tp_{ck}")
                dummy_mm(tp, eye_r)
                for i, bt in enumerate((2 * ck, 2 * ck + 1)):
                    tpb = tp.bitcast(BF16)[:, 64 * i:64 * i + 64]
                    nc.tensor.transpose(
                        tpb, in_=vT[64:128, bt * 128:(bt + 1) * 128],
                        identity=eye_sb[64:128, 64:128])
                    nc.scalar.copy(out=vones[:, bt, 0:64], in_=tpb)

            def emit_k2_repl(b):
                half = slice(b * S, (b + 1) * S)
                nc.sync.dma_start(out=k2[64:128, half], in_=k2[0:64, half])
                # ones column for this batch's vones tiles (ACT: scale 0)
                nc.scalar.activation(
                    out=vones[:, b * 16:(b + 1) * 16, 64:65],
                    in_=prep_sb[:, b, :, 0],
                    func=AF.Identity, bias=1.0, scale=0.0)
                # spend the k2-replication DMA wait on the PE clock
                dps = sps.tile([128, 1024], F32, tag="scores", name=f"k2d_{b}")
                nc.tensor.matmul(dps[0:1, 0:1],
                                 k2[64:128, b * S:b * S + 2].bitcast(F32)[:, 0:1],
                                 k2[64:128, b * S:b * S + 2].bitcast(F32)[:, 0:1],
                                 start=True, stop=True)

            ctxT_tiles = {}
            exs_map = {}

            def emit_att_sc(b, qh, g):
                q0 = b * S + qh * 1024
                if g == 0:
                    # No slot pre-spend needed: the first writer (the DVE
                    # tensor_copy) and the slot releaser (PE o_proj reads)
                    # pair with the PE-side transpose producer wait.
                    ctxT = [csb.tile([128, 1024], BF16, tag=f"ctxT{ft}",
                                     name=f"ctxT{ft}_{b}_{qh}") for ft in range(2)]
                    ctxT_tiles[(b, qh)] = ctxT
                ctxT = ctxT_tiles[(b, qh)]
                qt = qT[g // 2]
                qp = (g % 2) * 64
                exs = []
                for t in range(16):
                    sc = sps.tile([128, 1024], F32, tag="scores",
                                  name=f"sc_{b}_{qh}_{g}_{t}")
                    for qc in range(2):
                        nc.tensor.matmul(
                            sc[:, qc * 512:(qc + 1) * 512],
                            k2[qp:qp + 64, b * S + t * 128:b * S + (t + 1) * 128],
                            qt[qp:qp + 64, q0 + qc * 512:q0 + (qc + 1) * 512],
                            start=True, stop=True,
                        )
                    ex = ebp.tile([128, 1024], BF16, tag="expT",
                                  name=f"ex_{b}_{qh}_{g}_{t}")
                    exs.append(ex)
                    if _exp_on_dve(t):
                        nc.vector.tensor_scalar(
                            out=ex.bitcast(I16), in0=sc,
                            scalar1=prep_sb[:, b, t, 2:3],
                            scalar2=prep_sb[:, b, t, 1:2],
                            op0=mybir.AluOpType.max,
                            op1=mybir.AluOpType.add,
                        )
                    else:
                        nc.scalar.activation(
                            out=ex, in_=sc, func=AF.Exp,
                            bias=prep_sb[:, b, t, 0:1],
                            scale=ACT_SCALE,
                        )
                exs_map[(b, qh, g)] = exs

            def emit_att_ctx(b, qh, g):
                # transposed context: per 128-token q subtile, accumulate
                # ctx[q, d|den] over all kv tiles with ex as the stationary
                # operand (65 charged rows per matmul), then normalize by
                # the per-partition denominator column and transpose back.
                # Emitted one block late so the PE never head-of-line waits
                # on this block's own exp results.
                ctxT = ctxT_tiles[(b, qh)]
                exs = exs_map.pop((b, qh, g))
                qp = (g % 2) * 64
                for qp2 in range(4):        # pairs of 128-token q subtiles
                    cn = dsb.tile([128, 128], BF16, tag="ctxn",
                                  name=f"cn_{b}_{qh}_{g}_{qp2}")
                    for i in range(2):
                        qs = 2 * qp2 + i
                        cx = xps.tile([128, 65], F32, tag="ctx",
                                      name=f"cx_{b}_{qh}_{g}_{qs}")
                        # wait-carrier: spend the ctx-slot WAR wait (DVE
                        # release) before the real t=0 accumulation start.
                        dummy_mm(cx, wo_r)
                        for t in range(16):
                            nc.tensor.matmul(
                                cx,
                                exs[t][:, qs * 128:(qs + 1) * 128],
                                vones[:, b * 16 + t, :],
                                start=(t == 0), stop=(t == 15),
                            )
                        rc = dsb.tile([128, 1], F32, tag="recip",
                                      name=f"rc_{b}_{qh}_{g}_{qs}")
                        nc.vector.reciprocal(out=rc, in_=cx[:, 64:65])
                        nc.vector.tensor_scalar(
                            out=cn[:, 64 * i:64 * i + 64], in0=cx[:, 0:64],
                            scalar1=rc, scalar2=None,
                            op0=mybir.AluOpType.mult,
                        )
                    # one transpose covers both subtiles: out rows 0:64 are
                    # subtile 2*qp2 features, rows 64:128 the other's
                    tq = ops_pool.tile([128, 512], F32, tag="ops",
                                       name=f"tq_{b}_{qh}_{g}_{qp2}")
                    dummy_mm(tq, eye_r)
                    tqb = tq.bitcast(BF16)[:, 0:128]
                    nc.tensor.transpose(tqb, in_=cn, identity=eye_sb)
                    for i in range(2):
                        qs = 2 * qp2 + i
                        nc.vector.tensor_copy(
                            out=ctxT[g // 2][qp:qp + 64, qs * 128:(qs + 1) * 128],
                            in_=tqb[64 * i:64 * i + 64, :],
                        )

            def emit_oproj(b, qh, qq, tail=False):
                ctxT = ctxT_tiles[(b, qh)]
                osb = osp.tile([128, H], BF16, tag="osb", name=f"osb_{b}_{qh}_{qq}")
                # pre-spend the osb slot-reuse wait (out-DMA done)
                nc.vector.memset(osb[0:1, 0:1], 0.0)
                for ec in range(4):
                    op = ops_pool.tile([128, 512], F32, tag="ops",
                                       name=f"op_{b}_{qh}_{qq}_{ec}")
                    # carrier takes the slot-release wait (mixed ACT/DVE
                    # releasers from interleaved proj drains)
                    dummy_mm(op, wo_r)
                    for ft in range(2):
                        nc.tensor.matmul(
                            op,
                            ctxT[ft][:, qq * 128:(qq + 1) * 128],
                            w_o[:, ft, ec * 512:(ec + 1) * 512],
                            start=(ft == 0), stop=(ft == 1),
                        )
                    if tail and ec % 2 == 1:
                        nc.scalar.copy(
                            out=osb[:, ec * 512:(ec + 1) * 512], in_=op)
                    else:
                        nc.vector.tensor_copy(
                            out=osb[:, ec * 512:(ec + 1) * 512], in_=op)
                nc.sync.dma_start(
                    out=out[b, qh * 1024 + qq * 128:qh * 1024 + (qq + 1) * 128, :],
                    in_=osb,
                )

            # ---- software-pipelined emission schedule ----
            emit_proj(0, startup=True)
            emit_late_consts()
            for ck in range(1, 8):
                emit_proj(ck, startup=True)
            emit_k2_repl(0)

            proj_pending = list(range(8, 16))
            op_pending = []
            att_blocks = [(b, qh, g) for b in range(B) for qh in range(2)
                          for g in range(QH_PER_CORE)]
            ctx_queue = []
            for i, (b, qh, g) in enumerate(att_blocks):
                emit_att_sc(b, qh, g)
                ctx_queue.append((b, qh, g))
                if len(ctx_queue) > 2:
                    done = ctx_queue.pop(0)
                    emit_att_ctx(*done)
                    if done[2] == QH_PER_CORE - 1:
                        op_pending.extend((done[0], done[1], qq) for qq in range(8))
                if i == 7:
                    # b=1 attention needs all projections + its K replica
                    while proj_pending:
                        emit_proj(proj_pending.pop(0))
                    emit_k2_repl(1)
                    continue
                if proj_pending:
                    emit_proj(proj_pending.pop(0))
                    n_op = 1
                else:
                    n_op = 3
                for _ in range(min(n_op, len(op_pending))):
                    emit_oproj(*op_pending.pop(0))
            for done in ctx_queue:
                emit_att_ctx(*done)
                if done[2] == QH_PER_CORE - 1:
                    op_pending.extend((done[0], done[1], qq) for qq in range(8))
            while op_pending:
                emit_oproj(*op_pending.pop(0), tail=True)
    nc.compile()
    return nc


def kernel(hidden_states, attention_mask, Wq, bq, Wk, bk, Wv, bv, Wo, bo):
    hidden_states = np.asarray(hidden_states, dtype=np.float32)
    attention_mask = np.asarray(attention_mask, dtype=np.float32)
    Wq = np.asarray(Wq, dtype=np.float32)
    Wk = np.asarray(Wk, dtype=np.float32)
    Wv = np.asarray(Wv, dtype=np.float32)
    Wo = np.asarray(Wo, dtype=np.float32)

    if "nc" not in _CACHE:
        _CACHE["nc"] = _build_program()
    nc = _CACHE["nc"]

    hsT = np.ascontiguousarray(
        hidden_states.reshape(TOK, H).T).astype(ml_dtypes.bfloat16)  # [H, B*S]
    maskp = np.ascontiguousarray(
        attention_mask.reshape(B, S // 128, 128).transpose(2, 0, 1))  # [128, B, 16]
    sb16 = A16 * (maskp + EXP_SHIFT) + B16CONST
    prep = np.stack([maskp + EXP_SHIFT, sb16, -sb16], axis=-1).astype(np.float32)
    prep = np.ascontiguousarray(prep)                     # [128, B, 16, 3]
    eye = np.eye(128, dtype=np.float32).astype(ml_dtypes.bfloat16)

    in_maps = []
    for c in range(N_CORES):
        wq = Wq[QF * c:QF * (c + 1)]          # [256, H]
        wk = Wk[D * c:D * (c + 1)]            # [64, H]
        wv = Wv[D * c:D * (c + 1)]            # [64, H]
        wqkvT = np.ascontiguousarray(
            np.concatenate([wq, wk, wv], axis=0).T).astype(ml_dtypes.bfloat16)
        woT = np.ascontiguousarray(
            Wo[:, QF * c:QF * (c + 1)].T).astype(ml_dtypes.bfloat16)          # [256, H]
        bq_c = bq[QF * c:QF * (c + 1)] * ALPHA
        bk_c = bk[D * c:D * (c + 1)] * BETA
        bv_c = bv[D * c:D * (c + 1)]
        bqkv_c = np.ascontiguousarray(
            np.concatenate([bq_c, bk_c, bv_c]).astype(np.float32)
            .reshape(3, 128).T)               # [128, 3]
        in_maps.append({
            "hsT": hsT, "wqkvT": wqkvT, "woT": woT,
            "bqkv": bqkv_c, "prep": prep, "eye": eye,
        })

    _CACHE["last_in_maps"] = in_maps
    res = bass_utils.run_bass_kernel_spmd(nc, in_maps, core_ids=list(range(N_CORES)))
    acc = np.zeros((B, S, H), dtype=np.float32)
    for c in range(N_CORES):
        acc += np.asarray(res.results[c]["out"], dtype=np.float32)
    acc += np.asarray(bo, dtype=np.float32)[None, None, :]
    return acc


# revision 38
# speedup vs baseline: 1.4894x; 1.0264x over previous
"""GQA attention kernel for Trainium2, sharded over 8 NeuronCores.

Sharding: tensor-parallel over heads. Core c owns kv-head c and q-heads
4c..4c+3 (rows 256c:256c+256 of Wq, rows 64c:64c+64 of Wk/Wv) and columns
256c:256c+256 of Wo. Each core computes a full-shape partial of the output
(o_proj column-parallel); the host sums the 8 partials (the all-reduce)
and adds bo.

Per-core kernel layout choices:
- hidden_states is passed transposed [H, B*S] in bf16 so QKV projections
  contract over the partition dim with one big contiguous DMA per 256-token
  chunk (descriptor-generation time is per-DMA).
- Q,K,V are produced transposed ([feature, token]) by the PE in bf16.
  Q,K carry a scale ALPHA/BETA folded into their PSUM drain so the scores
  PSUM lands directly in Schraudolph-exp units (see below).
- Scores are computed transposed, S^T[t, q] = K_d,t^T . Q_d,q, so the
  softmax mask/shift folds into the exp bias (per-partition), and a
  ones-column appended to V yields softmax denominators as row 64 of the
  context matmul output.
- The softmax exp is engine-split: ACT runs the Exp activation (bf16 out)
  for 12/16 kv tiles; DVE computes the other 4/16 with one tensor_scalar
  op via the Schraudolph bit trick targeted at bf16:
  bits16(exp(z)) ~= (128/ln2) z + 16249.6, and ALPHA*BETA is chosen so the
  scores PSUM already holds (128/ln2) * logit. out_i16 = max(psum, -b) + b
  with b = (128/ln2)(mask-2) + 16249.6; the -2 shift cancels in the
  normalize.
- Context is accumulated TRANSPOSED: ctx[q, d] tiles [128, 65] with the
  just-computed ex tile as the PE stationary operand and V(+ones) as the
  64+1-column moving operand — 65 charged rows per matmul instead of 512
  (the cost model charges moving rows only), halving context PE time.
  The denominator lands in column 64 (per-partition), so normalization is
  a reciprocal plus one per-partition tensor_scalar multiply on DVE, and
  the normalized tile is PE-transposed back to feature-major for o_proj.
- Emission is software-pipelined: after a startup phase that projects the
  first batch's tokens, the remaining projection chunks, V-transposes and
  all o_proj blocks are interleaved between attention g-blocks so the PE
  (the bottleneck engine) never drains. Projection/o_proj/transpose PSUM
  tiles share the two "ops" PSUM bank slots; tiny [1,1] wait-carrier
  matmuls pre-spend slot-reuse semaphore waits so every real PE
  instruction needs at most its one producer wait.
- o_proj uses f32r matmuls (full rate at N=512); output is drained to
  bf16 by DVE and DMA'd at half traffic; the host sums partials in fp32.
"""

import os
import sys

for _p in ("/opt/trn_rl_repo",):
    if _p not in sys.path and os.path.isdir(_p):
        sys.path.insert(0, _p)

import numpy as np
import ml_dtypes

import concourse.bass as bass
import concourse.bacc as bacc
import concourse.tile as tile
from concourse import mybir
from concourse import bass_utils

F32 = mybir.dt.float32
F32R = mybir.dt.float32r
BF16 = mybir.dt.bfloat16
I16 = mybir.dt.int16
AF = mybir.ActivationFunctionType

B = 2
S = 2048
H = 2048
D = 64
N_CORES = 8
QH_PER_CORE = 4          # q-heads per core
QF = QH_PER_CORE * D     # 256 q features per core
TOK = B * S              # 4096
SCALE = 1.0 / np.sqrt(D)  # 0.125
CK = 256                 # projection chunk tokens

# bf16 Schraudolph-exp calibration. scores_psum = ALPHA*BETA*(q.k) = A16*s
# where s = SCALE*(q.k) is the true logit and A16 = 128/ln2 is the bf16
# bits-per-logit slope.
A16 = 128.0 / np.log(2.0)                # 184.664
AB = A16 * SCALE                         # required ALPHA*BETA
ALPHA = float(np.sqrt(AB))               # q drain scale (4.804)
BETA = float(np.sqrt(AB))                # k drain scale
EXP_SHIFT = -2.0                         # exp(s + mask - 2) range centering
C16 = 0.058 * 128.0                      # Schraudolph mean-shift
B16CONST = 127.0 * 128.0 - C16 + 0.5     # +0.5: astype(int16) truncates
ACT_SCALE = float(1.0 / A16)             # ACT-path exp input scale

_CACHE = {}


def _exp_on_dve(b, qh, g, t):
    """Which kv tiles the DVE (vs ACT) computes exp for: 4/16 so ACT:DVE
    engine busy stays balanced against their other work."""
    return t % 4 == 3


def _build_program():
    nc = bacc.Bacc("TRN2", target_bir_lowering=False, debug=False)

    hsT = nc.dram_tensor("hsT", [H, TOK], BF16, kind="ExternalInput").ap()
    wqkvT = nc.dram_tensor("wqkvT", [H, 384], BF16, kind="ExternalInput").ap()
    woT = nc.dram_tensor("woT", [QF, H], BF16, kind="ExternalInput").ap()
    bqkv = nc.dram_tensor("bqkv", [128, 3], F32, kind="ExternalInput").ap()
    prep = nc.dram_tensor("prep", [128, B, S // 128, 3], F32, kind="ExternalInput").ap()
    eye = nc.dram_tensor("eye", [128, 128], BF16, kind="ExternalInput").ap()
    out = nc.dram_tensor("out", [B, S, H], BF16, kind="ExternalOutput").ap()

    with tile.TileContext(nc) as tc:
        with tc.tile_pool(name="const", bufs=1) as cp, \
             tc.tile_pool(name="proj_sb", bufs=3) as psb, \
             tc.tile_pool(name="exb_sb", bufs=34) as ebp, \
             tc.tile_pool(name="drain_sb", bufs=3) as dsb, \
             tc.tile_pool(name="osb_sb", bufs=3) as osp, \
             tc.tile_pool(name="ctxT_sb", bufs=3) as csb, \
             tc.tile_pool(name="scores_ps", bufs=2, space="PSUM") as sps, \
             tc.tile_pool(name="ctx_ps", bufs=2, space="PSUM") as xps, \
             tc.tile_pool(name="o_ps", bufs=2, space="PSUM") as ops_pool:
            w_qkv = cp.tile([128, 16, 384], BF16)     # (p, h_tile, feature)
            nc.sync.dma_start(out=w_qkv, in_=wqkvT.rearrange("(t p) f -> p t f", p=128))
            bqkv_sb = cp.tile([128, 3], F32)
            nc.sync.dma_start(out=bqkv_sb, in_=bqkv)
            eye_sb = cp.tile([128, 128], BF16)
            nc.sync.dma_start(out=eye_sb, in_=eye)
            # w_o and prep are first consumed tens of microseconds in; issue
            # their DMAs behind the first projection chunk's token loads.
            w_o = cp.tile([128, 2, H], BF16)          # (p, f_tile, e)
            prep_sb = cp.tile([128, B, S // 128, 3], F32)

            def emit_late_consts():
                nc.sync.dma_start(out=w_o, in_=woT.rearrange("(t p) e -> p t e", p=128))
                nc.sync.dma_start(out=prep_sb, in_=prep)

            # Engine wait budgets are tiny (1 sync-wait per instruction for
            # PE/ACT structs). Warm consumer-engine vector clocks on the
            # small const DMAs so real instructions never need extra waits.
            scratch = cp.tile([128, 1], F32)
            nc.scalar.copy(out=scratch, in_=bqkv_sb[:, 0:1])
            nc.scalar.copy(out=scratch, in_=prep_sb[:, 0, 0, 0:1])
            scratch_d = cp.tile([128, 1], F32)
            nc.vector.tensor_copy(out=scratch_d, in_=prep_sb[:, 0, 0, 1:2])

            # Q^T bf16 scaled by ALPHA (2 q-heads per tile), V^T bf16
            # (partitions 64:128), K^T bf16 scaled by BETA replicated in
            # both partition halves (PE base-partition alignment for the
            # per-head scores matmuls).
            qT = [cp.tile([128, TOK], BF16, name=f"qT{i}") for i in range(2)]
            vT = cp.tile([128, TOK], BF16)
            k2 = cp.tile([128, TOK], BF16)
            # V transposed back to [t, d] + ones column, per 128-token tile.
            vones = cp.tile([128, B * 16, 65], BF16)

            hsT_tiled = hsT.rearrange("(t p) n -> p t n", p=128)

            def dummy_mm(ps_tile, reader):
                """[1,1] wait-carrier matmul: first touch of a PSUM slot,
                spending its slot-reuse wait; `reader` must be an SBUF AP
                whose producer clock the PE already knows."""
                nc.tensor.matmul(ps_tile[0:1, 0:1], reader, reader,
                                 start=True, stop=True)

            w_r = w_qkv[:, 0, 0:2].bitcast(F32)[:, 0:1]
            wo_r = w_o[:, 0, 0:2].bitcast(F32)[:, 0:1]
            eye_r = eye_sb[0:64, 0:2].bitcast(F32)[:, 0:1]

            def emit_proj(ck, startup=False):
                hstage = psb.tile([128, 16, CK], BF16, tag="hstage", name=f"hs_{ck}")
                nc.sync.dma_start(
                    out=hstage, in_=hsT_tiled[:, :, ck * CK:(ck + 1) * CK])
                cols = slice(ck * CK, (ck + 1) * CK)
                for ft in range(3):
                    # during startup the attention ctx PSUM slots are idle;
                    # alternating pools doubles the slot-recycle distance
                    pool = xps if startup and ft == 1 else ops_pool
                    ps = pool.tile([128, 512], F32, tag="ctx" if pool is xps else "ops",
                                   name=f"pj_{ck}_{ft}")
                    # carrier takes the slot-release wait; the first real
                    # matmul then only waits on the hstage DMA.
                    dummy_mm(ps, w_r)
                    for ht in range(16):
                        nc.tensor.matmul(
                            ps[:, 0:CK],
                            w_qkv[:, ht, ft * 128:(ft + 1) * 128],
                            hstage[:, ht, :],
                            start=(ht == 0), stop=(ht == 15),
                        )
                    if ft < 2:
                        nc.scalar.activation(
                            out=qT[ft][:, cols], in_=ps[:, 0:CK],
                            func=AF.Identity, bias=bqkv_sb[:, ft:ft + 1],
                            scale=ALPHA,
                        )
                    else:
                        nc.scalar.activation(
                            out=k2[0:64, cols], in_=ps[0:64, 0:CK],
                            func=AF.Identity, bias=bqkv_sb[0:64, 2:3],
                            scale=BETA,
                        )
                        nc.scalar.activation(
                            out=vT[64:128, cols], in_=ps[64:128, 0:CK],
                            func=AF.Identity, bias=bqkv_sb[64:128, 2:3],
                        )
                # V[t, d] tiles for the two 128-token tiles of this chunk
                # (both share one ops slot at different bf16 column ranges)
                pool = xps if startup else ops_pool
                tp = pool.tile([128, 512], F32, tag="ctx" if startup else "ops",
                               name=f"tp_{ck}")
                dummy_mm(tp, eye_r)
                for i, bt in enumerate((2 * ck, 2 * ck + 1)):
                    tpb = tp.bitcast(BF16)[:, 64 * i:64 * i + 64]
                    nc.tensor.transpose(
                        tpb, in_=vT[64:128, bt * 128:(bt + 1) * 128],
                        identity=eye_sb[64:128, 64:128])
                    nc.scalar.copy(out=vones[:, bt, 0:64], in_=tpb)

            def emit_k2_repl(b):
                half = slice(b * S, (b + 1) * S)
                nc.sync.dma_start(out=k2[64:128, half], in_=k2[0:64, half])
                # ones column for this batch's vones tiles (ACT: scale 0)
                nc.scalar.activation(
                    out=vones[:, b * 16:(b + 1) * 16, 64:65],
                    in_=prep_sb[:, b, :, 0],
                    func=AF.Identity, bias=1.0, scale=0.0)
                # spend the k2-replication DMA wait on the PE clock
                dps = sps.tile([128, 1024], F32, tag="scores", name=f"k2d_{b}")
                nc.tensor.matmul(dps[0:1, 0:1],
                                 k2[64:128, b * S:b * S + 2].bitcast(F32)[:, 0:1],
                                 k2[64:128, b * S:b * S + 2].bitcast(F32)[:, 0:1],
                                 start=True, stop=True)

            ctxT_tiles = {}
            exs_map = {}

            def emit_att_sc(b, qh, g):
                q0 = b * S + qh * 1024
                if g == 0:
                    # No slot pre-spend needed: the first writer (the DVE
                    # tensor_copy) and the slot releaser (PE o_proj reads)
                    # pair with the PE-side transpose producer wait.
                    ctxT = [csb.tile([128, 1024], BF16, tag=f"ctxT{ft}",
                                     name=f"ctxT{ft}_{b}_{qh}") for ft in range(2)]
                    ctxT_tiles[(b, qh)] = ctxT
                ctxT = ctxT_tiles[(b, qh)]
                qt = qT[g // 2]
                qp = (g % 2) * 64
                exs = []
                for t in range(16):
                    sc = sps.tile([128, 1024], F32, tag="scores",
                                  name=f"sc_{b}_{qh}_{g}_{t}")
                    for qc in range(2):
                        nc.tensor.matmul(
                            sc[:, qc * 512:(qc + 1) * 512],
                            k2[qp:qp + 64, b * S + t * 128:b * S + (t + 1) * 128],
                            qt[qp:qp + 64, q0 + qc * 512:q0 + (qc + 1) * 512],
                            start=True, stop=True,
                        )
                    ex = ebp.tile([128, 1024], BF16, tag="expT",
                                  name=f"ex_{b}_{qh}_{g}_{t}")
                    exs.append(ex)
                    if _exp_on_dve(b, qh, g, t):
                        nc.vector.tensor_scalar(
                            out=ex.bitcast(I16), in0=sc,
                            scalar1=prep_sb[:, b, t, 2:3],
                            scalar2=prep_sb[:, b, t, 1:2],
                            op0=mybir.AluOpType.max,
                            op1=mybir.AluOpType.add,
                        )
                    else:
                        nc.scalar.activation(
                            out=ex, in_=sc, func=AF.Exp,
                            bias=prep_sb[:, b, t, 0:1],
                            scale=ACT_SCALE,
                        )
                exs_map[(b, qh, g)] = exs

            def emit_att_ctx(b, qh, g):
                # transposed context: per 128-token q subtile, accumulate
                # ctx[q, d|den] over all kv tiles with ex as the stationary
                # operand (65 charged rows per matmul), then normalize by
                # the per-partition denominator column and transpose back.
                # Emitted one block late so the PE never head-of-line waits
                # on this block's own exp results.
                ctxT = ctxT_tiles[(b, qh)]
                exs = exs_map.pop((b, qh, g))
                qp = (g % 2) * 64
                for qp2 in range(4):        # pairs of 128-token q subtiles
                    cn = dsb.tile([128, 128], BF16, tag="ctxn",
                                  name=f"cn_{b}_{qh}_{g}_{qp2}")
                    for i in range(2):
                        qs = 2 * qp2 + i
                        cx = xps.tile([128, 65], F32, tag="ctx",
                                      name=f"cx_{b}_{qh}_{g}_{qs}")
                        # wait-carrier: spend the ctx-slot WAR wait (DVE
                        # release) before the real t=0 accumulation start.
                        dummy_mm(cx, wo_r)
                        for t in range(16):
                            nc.tensor.matmul(
                                cx,
                                exs[t][:, qs * 128:(qs + 1) * 128],
                                vones[:, b * 16 + t, :],
                                start=(t == 0), stop=(t == 15),
                            )
                        rc = dsb.tile([128, 1], F32, tag="recip",
                                      name=f"rc_{b}_{qh}_{g}_{qs}")
                        nc.vector.reciprocal(out=rc, in_=cx[:, 64:65])
                        nc.vector.tensor_scalar(
                            out=cn[:, 64 * i:64 * i + 64], in0=cx[:, 0:64],
                            scalar1=rc, scalar2=None,
                            op0=mybir.AluOpType.mult,
                        )
                    # one transpose covers both subtiles: out rows 0:64 are
                    # subtile 2*qp2 features, rows 64:128 the other's
                    tq = ops_pool.tile([128, 512], F32, tag="ops",
                                       name=f"tq_{b}_{qh}_{g}_{qp2}")
                    dummy_mm(tq, eye_r)
                    tqb = tq.bitcast(BF16)[:, 0:128]
                    nc.tensor.transpose(tqb, in_=cn, identity=eye_sb)
                    for i in range(2):
                        qs = 2 * qp2 + i
                        nc.vector.tensor_copy(
                            out=ctxT[g // 2][qp:qp + 64, qs * 128:(qs + 1) * 128],
                            in_=tqb[64 * i:64 * i + 64, :],
                        )

            def emit_oproj(b, qh, qq, tail=False):
                ctxT = ctxT_tiles[(b, qh)]
                osb = osp.tile([128, H], BF16, tag="osb", name=f"osb_{b}_{qh}_{qq}")
                # pre-spend the osb slot-reuse wait (out-DMA done)
                nc.vector.memset(osb[0:1, 0:1], 0.0)
                for ec in range(4):
                    op = ops_pool.tile([128, 512], F32, tag="ops",
                                       name=f"op_{b}_{qh}_{qq}_{ec}")
                    # carrier takes the slot-release wait (mixed ACT/DVE
                    # releasers from interleaved proj drains)
                    dummy_mm(op, wo_r)
                    for ft in range(2):
                        nc.tensor.matmul(
                            op,
                            ctxT[ft][:, qq * 128:(qq + 1) * 128],
                            w_o[:, ft, ec * 512:(ec + 1) * 512],
                            start=(ft == 0), stop=(ft == 1),
                        )
                    if tail and ec % 2 == 1:
                        nc.scalar.copy(
                            out=osb[:, ec * 512:(ec + 1) * 512], in_=op)
                    else:
                        nc.vector.tensor_copy(
                            out=osb[:, ec * 512:(ec + 1) * 512], in_=op)
                nc.sync.dma_start(
                    out=out[b, qh * 1024 + qq * 128:qh * 1024 + (qq + 1) * 128, :],
                    in_=osb,
                )

            # ---- software-pipelined emission schedule ----
            emit_proj(0, startup=True)
            emit_late_consts()
            for ck in range(1, 8):
                emit_proj(ck, startup=True)
            emit_k2_repl(0)

            proj_pending = list(range(8, 16))
            op_pending = []
            att_blocks = [(b, qh, g) for b in range(B) for qh in range(2)
                          for g in range(QH_PER_CORE)]
            ctx_queue = []
            for i, (b, qh, g) in enumerate(att_blocks):
                emit_att_sc(b, qh, g)
                ctx_queue.append((b, qh, g))
                if len(ctx_queue) > 2:
                    done = ctx_queue.pop(0)
                    emit_att_ctx(*done)
                    if done[2] == QH_PER_CORE - 1:
                        op_pending.extend((done[0], done[1], qq) for qq in range(8))
                if i == 7:
                    # b=1 attention needs all projections + its K replica
                    while proj_pending:
                        emit_proj(proj_pending.pop(0))
                    emit_k2_repl(1)
                    continue
                if proj_pending:
                    emit_proj(proj_pending.pop(0))
                    n_op = 1
                else:
                    n_op = 3
                for _ in range(min(n_op, len(op_pending))):
                    emit_oproj(*op_pending.pop(0))
            for done in ctx_queue:
                emit_att_ctx(*done)
                if done[2] == QH_PER_CORE - 1:
                    op_pending.extend((done[0], done[1], qq) for qq in range(8))
            while op_pending:
                emit_oproj(*op_pending.pop(0), tail=True)
    nc.compile()
    return nc


def kernel(hidden_states, attention_mask, Wq, bq, Wk, bk, Wv, bv, Wo, bo):
    hidden_states = np.asarray(hidden_states, dtype=np.float32)
    attention_mask = np.asarray(attention_mask, dtype=np.float32)
    Wq = np.asarray(Wq, dtype=np.float32)
    Wk = np.asarray(Wk, dtype=np.float32)
    Wv = np.asarray(Wv, dtype=np.float32)
    Wo = np.asarray(Wo, dtype=np.float32)

    if "nc" not in _CACHE:
        _CACHE["nc"] = _build_program()
    nc = _CACHE["nc"]

    hsT = np.ascontiguousarray(
        hidden_states.reshape(TOK, H).T).astype(ml_dtypes.bfloat16)  # [H, B*S]
    maskp = np.ascontiguousarray(
        attention_mask.reshape(B, S // 128, 128).transpose(2, 0, 1))  # [128, B, 16]
    sb16 = A16 * (maskp + EXP_SHIFT) + B16CONST
    prep = np.stack([maskp + EXP_SHIFT, sb16, -sb16], axis=-1).astype(np.float32)
    prep = np.ascontiguousarray(prep)                     # [128, B, 16, 3]
    eye = np.eye(128, dtype=np.float32).astype(ml_dtypes.bfloat16)

    in_maps = []
    for c in range(N_CORES):
        wq = Wq[QF * c:QF * (c + 1)]          # [256, H]
        wk = Wk[D * c:D * (c + 1)]            # [64, H]
        wv = Wv[D * c:D * (c + 1)]            # [64, H]
        wqkvT = np.ascontiguousarray(
            np.concatenate([wq, wk, wv], axis=0).T).astype(ml_dtypes.bfloat16)
        woT = np.ascontiguousarray(
            Wo[:, QF * c:QF * (c + 1)].T).astype(ml_dtypes.bfloat16)          # [256, H]
        bq_c = bq[QF * c:QF * (c + 1)] * ALPHA
        bk_c = bk[D * c:D * (c + 1)] * BETA
        bv_c = bv[D * c:D * (c + 1)]
        bqkv_c = np.ascontiguousarray(
            np.concatenate([bq_c, bk_c, bv_c]).astype(np.float32)
            .reshape(3, 128).T)               # [128, 3]
        in_maps.append({
            "hsT": hsT, "wqkvT": wqkvT, "woT": woT,
            "bqkv": bqkv_c, "prep": prep, "eye": eye,
        })

    _CACHE["last_in_maps"] = in_maps
    res = bass_utils.run_bass_kernel_spmd(nc, in_maps, core_ids=list(range(N_CORES)))
    acc = np.zeros((B, S, H), dtype=np.float32)
    for c in range(N_CORES):
        acc += np.asarray(res.results[c]["out"], dtype=np.float32)
    acc += np.asarray(bo, dtype=np.float32)[None, None, :]
    return acc


# revision 40
# speedup vs baseline: 1.4909x; 1.0011x over previous
"""GQA attention kernel for Trainium2, sharded over 8 NeuronCores.

Sharding: tensor-parallel over heads. Core c owns kv-head c and q-heads
4c..4c+3 (rows 256c:256c+256 of Wq, rows 64c:64c+64 of Wk/Wv) and columns
256c:256c+256 of Wo. Each core computes a full-shape partial of the output
(o_proj column-parallel); the host sums the 8 partials (the all-reduce)
and adds bo.

Per-core kernel layout choices:
- hidden_states is passed transposed [H, B*S] in bf16 so QKV projections
  contract over the partition dim with one big contiguous DMA per 256-token
  chunk (descriptor-generation time is per-DMA).
- Q,K,V are produced transposed ([feature, token]) by the PE in bf16.
  Q,K carry a scale ALPHA/BETA folded into their PSUM drain so the scores
  PSUM lands directly in Schraudolph-exp units (see below).
- Scores are computed transposed, S^T[t, q] = K_d,t^T . Q_d,q, so the
  softmax mask/shift folds into the exp bias (per-partition), and a
  ones-column appended to V yields softmax denominators as row 64 of the
  context matmul output.
- The softmax exp is engine-split: ACT runs the Exp activation (bf16 out)
  for 12/16 kv tiles; DVE computes the other 4/16 with one tensor_scalar
  op via the Schraudolph bit trick targeted at bf16:
  bits16(exp(z)) ~= (128/ln2) z + 16249.6, and ALPHA*BETA is chosen so the
  scores PSUM already holds (128/ln2) * logit. out_i16 = max(psum, -b) + b
  with b = (128/ln2)(mask-2) + 16249.6; the -2 shift cancels in the
  normalize.
- Context is accumulated TRANSPOSED: ctx[q, d] tiles [128, 65] with the
  just-computed ex tile as the PE stationary operand and V(+ones) as the
  64+1-column moving operand — 65 charged rows per matmul instead of 512
  (the cost model charges moving rows only), halving context PE time.
  The denominator lands in column 64 (per-partition), so normalization is
  a reciprocal plus one per-partition tensor_scalar multiply on DVE, and
  the normalized tile is PE-transposed back to feature-major for o_proj.
- Emission is software-pipelined: after a startup phase that projects the
  first batch's tokens, the remaining projection chunks, V-transposes and
  all o_proj blocks are interleaved between attention g-blocks so the PE
  (the bottleneck engine) never drains. Projection/o_proj/transpose PSUM
  tiles share the two "ops" PSUM bank slots; tiny [1,1] wait-carrier
  matmuls pre-spend slot-reuse semaphore waits so every real PE
  instruction needs at most its one producer wait.
- o_proj uses f32r matmuls (full rate at N=512); output is drained to
  bf16 by DVE and DMA'd at half traffic; the host sums partials in fp32.
"""

import os
import sys

for _p in ("/opt/trn_rl_repo",):
    if _p not in sys.path and os.path.isdir(_p):
        sys.path.insert(0, _p)

import numpy as np
import ml_dtypes

import concourse.bass as bass
import concourse.bacc as bacc
import concourse.tile as tile
from concourse import mybir
from concourse import bass_utils

F32 = mybir.dt.float32
F32R = mybir.dt.float32r
BF16 = mybir.dt.bfloat16
I16 = mybir.dt.int16
AF = mybir.ActivationFunctionType

B = 2
S = 2048
H = 2048
D = 64
N_CORES = 8
QH_PER_CORE = 4          # q-heads per core
QF = QH_PER_CORE * D     # 256 q features per core
TOK = B * S              # 4096
SCALE = 1.0 / np.sqrt(D)  # 0.125
CK = 256                 # projection chunk tokens

# bf16 Schraudolph-exp calibration. scores_psum = ALPHA*BETA*(q.k) = A16*s
# where s = SCALE*(q.k) is the true logit and A16 = 128/ln2 is the bf16
# bits-per-logit slope.
A16 = 128.0 / np.log(2.0)                # 184.664
AB = A16 * SCALE                         # required ALPHA*BETA
ALPHA = float(np.sqrt(AB))               # q drain scale (4.804)
BETA = float(np.sqrt(AB))                # k drain scale
EXP_SHIFT = -2.0                         # exp(s + mask - 2) range centering
C16 = 0.058 * 128.0                      # Schraudolph mean-shift
B16CONST = 127.0 * 128.0 - C16 + 0.5     # +0.5: astype(int16) truncates
ACT_SCALE = float(1.0 / A16)             # ACT-path exp input scale

_CACHE = {}


def _exp_on_dve(b, qh, g, t):
    """Which kv tiles the DVE (vs ACT) computes exp for: 4/16 so ACT:DVE
    engine busy stays balanced against their other work. The first two
    blocks lean on DVE while ACT clears the projection-drain backlog."""
    if b == 0 and qh == 0 and g < 2:
        return t % 2 == 1
    return t % 4 == 3


def _build_program():
    nc = bacc.Bacc("TRN2", target_bir_lowering=False, debug=False)

    hsT = nc.dram_tensor("hsT", [H, TOK], BF16, kind="ExternalInput").ap()
    wqkvT = nc.dram_tensor("wqkvT", [H, 384], BF16, kind="ExternalInput").ap()
    woT = nc.dram_tensor("woT", [QF, H], BF16, kind="ExternalInput").ap()
    bqkv = nc.dram_tensor("bqkv", [128, 3], F32, kind="ExternalInput").ap()
    prep = nc.dram_tensor("prep", [128, B, S // 128, 3], F32, kind="ExternalInput").ap()
    eye = nc.dram_tensor("eye", [128, 128], BF16, kind="ExternalInput").ap()
    out = nc.dram_tensor("out", [B, S, H], BF16, kind="ExternalOutput").ap()

    with tile.TileContext(nc) as tc:
        with tc.tile_pool(name="const", bufs=1) as cp, \
             tc.tile_pool(name="proj_sb", bufs=3) as psb, \
             tc.tile_pool(name="exb_sb", bufs=34) as ebp, \
             tc.tile_pool(name="drain_sb", bufs=3) as dsb, \
             tc.tile_pool(name="osb_sb", bufs=3) as osp, \
             tc.tile_pool(name="ctxT_sb", bufs=3) as csb, \
             tc.tile_pool(name="scores_ps", bufs=2, space="PSUM") as sps, \
             tc.tile_pool(name="ctx_ps", bufs=2, space="PSUM") as xps, \
             tc.tile_pool(name="o_ps", bufs=2, space="PSUM") as ops_pool:
            w_qkv = cp.tile([128, 16, 384], BF16)     # (p, h_tile, feature)
            wq_t = wqkvT.rearrange("(t p) f -> p t f", p=128)
            nc.sync.dma_start(out=w_qkv[:, 0:8, :], in_=wq_t[:, 0:8, :])
            nc.sync.dma_start(out=w_qkv[:, 8:16, :], in_=wq_t[:, 8:16, :])
            bqkv_sb = cp.tile([128, 3], F32)
            nc.sync.dma_start(out=bqkv_sb, in_=bqkv)
            eye_sb = cp.tile([128, 128], BF16)
            nc.sync.dma_start(out=eye_sb, in_=eye)
            # w_o and prep are first consumed tens of microseconds in; issue
            # their DMAs behind the first projection chunk's token loads.
            w_o = cp.tile([128, 2, H], BF16)          # (p, f_tile, e)
            prep_sb = cp.tile([128, B, S // 128, 3], F32)

            def emit_late_consts():
                nc.sync.dma_start(out=w_o, in_=woT.rearrange("(t p) e -> p t e", p=128))
                nc.sync.dma_start(out=prep_sb, in_=prep)

            # Engine wait budgets are tiny (1 sync-wait per instruction for
            # PE/ACT structs). Warm consumer-engine vector clocks on the
            # small const DMAs so real instructions never need extra waits.
            scratch = cp.tile([128, 1], F32)
            nc.scalar.copy(out=scratch, in_=bqkv_sb[:, 0:1])
            nc.scalar.copy(out=scratch, in_=prep_sb[:, 0, 0, 0:1])
            scratch_d = cp.tile([128, 1], F32)
            nc.vector.tensor_copy(out=scratch_d, in_=prep_sb[:, 0, 0, 1:2])

            # Q^T bf16 scaled by ALPHA (2 q-heads per tile), V^T bf16
            # (partitions 64:128), K^T bf16 scaled by BETA replicated in
            # both partition halves (PE base-partition alignment for the
            # per-head scores matmuls).
            qT = [cp.tile([128, TOK], BF16, name=f"qT{i}") for i in range(2)]
            vT = cp.tile([128, TOK], BF16)
            k2 = cp.tile([128, TOK], BF16)
            # V transposed back to [t, d] + ones column, per 128-token tile.
            vones = cp.tile([128, B * 16, 65], BF16)

            hsT_tiled = hsT.rearrange("(t p) n -> p t n", p=128)

            def dummy_mm(ps_tile, reader):
                """[1,1] wait-carrier matmul: first touch of a PSUM slot,
                spending its slot-reuse wait; `reader` must be an SBUF AP
                whose producer clock the PE already knows."""
                nc.tensor.matmul(ps_tile[0:1, 0:1], reader, reader,
                                 start=True, stop=True)

            w_r = w_qkv[:, 0, 0:2].bitcast(F32)[:, 0:1]
            wo_r = w_o[:, 0, 0:2].bitcast(F32)[:, 0:1]
            eye_r = eye_sb[0:64, 0:2].bitcast(F32)[:, 0:1]

            def emit_proj(ck, startup=False):
                hstage = psb.tile([128, 16, CK], BF16, tag="hstage", name=f"hs_{ck}")
                nc.sync.dma_start(
                    out=hstage, in_=hsT_tiled[:, :, ck * CK:(ck + 1) * CK])
                cols = slice(ck * CK, (ck + 1) * CK)
                for ft in range(3):
                    # during startup the attention ctx PSUM slots are idle;
                    # alternating pools doubles the slot-recycle distance
                    pool = xps if startup and ft == 1 else ops_pool
                    ps = pool.tile([128, 512], F32, tag="ctx" if pool is xps else "ops",
                                   name=f"pj_{ck}_{ft}")
                    # carrier takes the slot-release wait; the first real
                    # matmul then only waits on the hstage DMA.
                    dummy_mm(ps, w_r)
                    for ht in range(16):
                        nc.tensor.matmul(
                            ps[:, 0:CK],
                            w_qkv[:, ht, ft * 128:(ft + 1) * 128],
                            hstage[:, ht, :],
                            start=(ht == 0), stop=(ht == 15),
                        )
                    if ft < 2:
                        nc.scalar.activation(
                            out=qT[ft][:, cols], in_=ps[:, 0:CK],
                            func=AF.Identity, bias=bqkv_sb[:, ft:ft + 1],
                            scale=ALPHA,
                        )
                    else:
                        nc.scalar.activation(
                            out=k2[0:64, cols], in_=ps[0:64, 0:CK],
                            func=AF.Identity, bias=bqkv_sb[0:64, 2:3],
                            scale=BETA,
                        )
                        nc.scalar.activation(
                            out=vT[64:128, cols], in_=ps[64:128, 0:CK],
                            func=AF.Identity, bias=bqkv_sb[64:128, 2:3],
                        )
                # V[t, d] tiles for the two 128-token tiles of this chunk
                # (both share one ops slot at different bf16 column ranges)
                pool = xps if startup else ops_pool
                tp = pool.tile([128, 512], F32, tag="ctx" if startup else "ops",
                               name=f"tp_{ck}")
                dummy_mm(tp, eye_r)
                for i, bt in enumerate((2 * ck, 2 * ck + 1)):
                    tpb = tp.bitcast(BF16)[:, 64 * i:64 * i + 64]
                    nc.tensor.transpose(
                        tpb, in_=vT[64:128, bt * 128:(bt + 1) * 128],
                        identity=eye_sb[64:128, 64:128])
                    nc.scalar.copy(out=vones[:, bt, 0:64], in_=tpb)

            def emit_k2_repl(b):
                half = slice(b * S, (b + 1) * S)
                nc.sync.dma_start(out=k2[64:128, half], in_=k2[0:64, half])
                # ones column for this batch's vones tiles (ACT: scale 0)
                nc.scalar.activation(
                    out=vones[:, b * 16:(b + 1) * 16, 64:65],
                    in_=prep_sb[:, b, :, 0],
                    func=AF.Identity, bias=1.0, scale=0.0)
                # spend the k2-replication DMA wait on the PE clock
                dps = sps.tile([128, 1024], F32, tag="scores", name=f"k2d_{b}")
                nc.tensor.matmul(dps[0:1, 0:1],
                                 k2[64:128, b * S:b * S + 2].bitcast(F32)[:, 0:1],
                                 k2[64:128, b * S:b * S + 2].bitcast(F32)[:, 0:1],
                                 start=True, stop=True)

            ctxT_tiles = {}
            exs_map = {}

            def emit_att_sc(b, qh, g):
                q0 = b * S + qh * 1024
                if g == 0:
                    # No slot pre-spend needed: the first writer (the DVE
                    # tensor_copy) and the slot releaser (PE o_proj reads)
                    # pair with the PE-side transpose producer wait.
                    ctxT = [csb.tile([128, 1024], BF16, tag=f"ctxT{ft}",
                                     name=f"ctxT{ft}_{b}_{qh}") for ft in range(2)]
                    ctxT_tiles[(b, qh)] = ctxT
                ctxT = ctxT_tiles[(b, qh)]
                qt = qT[g // 2]
                qp = (g % 2) * 64
                exs = []
                for t in range(16):
                    sc = sps.tile([128, 1024], F32, tag="scores",
                                  name=f"sc_{b}_{qh}_{g}_{t}")
                    for qc in range(2):
                        nc.tensor.matmul(
                            sc[:, qc * 512:(qc + 1) * 512],
                            k2[qp:qp + 64, b * S + t * 128:b * S + (t + 1) * 128],
                            qt[qp:qp + 64, q0 + qc * 512:q0 + (qc + 1) * 512],
                            start=True, stop=True,
                        )
                    ex = ebp.tile([128, 1024], BF16, tag="expT",
                                  name=f"ex_{b}_{qh}_{g}_{t}")
                    exs.append(ex)
                    if _exp_on_dve(b, qh, g, t):
                        nc.vector.tensor_scalar(
                            out=ex.bitcast(I16), in0=sc,
                            scalar1=prep_sb[:, b, t, 2:3],
                            scalar2=prep_sb[:, b, t, 1:2],
                            op0=mybir.AluOpType.max,
                            op1=mybir.AluOpType.add,
                        )
                    else:
                        nc.scalar.activation(
                            out=ex, in_=sc, func=AF.Exp,
                            bias=prep_sb[:, b, t, 0:1],
                            scale=ACT_SCALE,
                        )
                exs_map[(b, qh, g)] = exs

            def emit_att_ctx(b, qh, g):
                # transposed context: per 128-token q subtile, accumulate
                # ctx[q, d|den] over all kv tiles with ex as the stationary
                # operand (65 charged rows per matmul), then normalize by
                # the per-partition denominator column and transpose back.
                # Emitted one block late so the PE never head-of-line waits
                # on this block's own exp results.
                ctxT = ctxT_tiles[(b, qh)]
                exs = exs_map.pop((b, qh, g))
                qp = (g % 2) * 64
                for qp2 in range(4):        # pairs of 128-token q subtiles
                    cn = dsb.tile([128, 128], BF16, tag="ctxn",
                                  name=f"cn_{b}_{qh}_{g}_{qp2}")
                    for i in range(2):
                        qs = 2 * qp2 + i
                        cx = xps.tile([128, 65], F32, tag="ctx",
                                      name=f"cx_{b}_{qh}_{g}_{qs}")
                        # wait-carrier: spend the ctx-slot WAR wait (DVE
                        # release) before the real t=0 accumulation start.
                        dummy_mm(cx, wo_r)
                        for t in range(16):
                            nc.tensor.matmul(
                                cx,
                                exs[t][:, qs * 128:(qs + 1) * 128],
                                vones[:, b * 16 + t, :],
                                start=(t == 0), stop=(t == 15),
                            )
                        rc = dsb.tile([128, 1], F32, tag="recip",
                                      name=f"rc_{b}_{qh}_{g}_{qs}")
                        nc.vector.reciprocal(out=rc, in_=cx[:, 64:65])
                        nc.vector.tensor_scalar(
                            out=cn[:, 64 * i:64 * i + 64], in0=cx[:, 0:64],
                            scalar1=rc, scalar2=None,
                            op0=mybir.AluOpType.mult,
                        )
                    # one transpose covers both subtiles: out rows 0:64 are
                    # subtile 2*qp2 features, rows 64:128 the other's
                    tq = ops_pool.tile([128, 512], F32, tag="ops",
                                       name=f"tq_{b}_{qh}_{g}_{qp2}")
                    dummy_mm(tq, eye_r)
                    tqb = tq.bitcast(BF16)[:, 0:128]
                    nc.tensor.transpose(tqb, in_=cn, identity=eye_sb)
                    for i in range(2):
                        qs = 2 * qp2 + i
                        nc.vector.tensor_copy(
                            out=ctxT[g // 2][qp:qp + 64, qs * 128:(qs + 1) * 128],
                            in_=tqb[64 * i:64 * i + 64, :],
                        )

            def emit_oproj(b, qh, qq, tail=False):
                ctxT = ctxT_tiles[(b, qh)]
                osb = osp.tile([128, H], BF16, tag="osb", name=f"osb_{b}_{qh}_{qq}")
                # pre-spend the osb slot-reuse wait (out-DMA done)
                nc.vector.memset(osb[0:1, 0:1], 0.0)
                for ec in range(4):
                    op = ops_pool.tile([128, 512], F32, tag="ops",
                                       name=f"op_{b}_{qh}_{qq}_{ec}")
                    # carrier takes the slot-release wait (mixed ACT/DVE
                    # releasers from interleaved proj drains)
                    dummy_mm(op, wo_r)
                    for ft in range(2):
                        nc.tensor.matmul(
                            op,
                            ctxT[ft][:, qq * 128:(qq + 1) * 128],
                            w_o[:, ft, ec * 512:(ec + 1) * 512],
                            start=(ft == 0), stop=(ft == 1),
                        )
                    if tail and ec % 2 == 1:
                        nc.scalar.copy(
                            out=osb[:, ec * 512:(ec + 1) * 512], in_=op)
                    else:
                        nc.vector.tensor_copy(
                            out=osb[:, ec * 512:(ec + 1) * 512], in_=op)
                nc.sync.dma_start(
                    out=out[b, qh * 1024 + qq * 128:qh * 1024 + (qq + 1) * 128, :],
                    in_=osb,
                )

            # ---- software-pipelined emission schedule ----
            emit_proj(0, startup=True)
            emit_late_consts()
            for ck in range(1, 8):
                emit_proj(ck, startup=True)
            emit_k2_repl(0)

            proj_pending = list(range(8, 16))
            op_pending = []
            att_blocks = [(b, qh, g) for b in range(B) for qh in range(2)
                          for g in range(QH_PER_CORE)]
            ctx_queue = []
            for i, (b, qh, g) in enumerate(att_blocks):
                emit_att_sc(b, qh, g)
                ctx_queue.append((b, qh, g))
                if len(ctx_queue) > 2:
                    done = ctx_queue.pop(0)
                    emit_att_ctx(*done)
                    if done[2] == QH_PER_CORE - 1:
                        op_pending.extend((done[0], done[1], qq) for qq in range(8))
                if i == 7:
                    # b=1 attention needs all projections + its K replica
                    while proj_pending:
                        emit_proj(proj_pending.pop(0))
                    emit_k2_repl(1)
                    continue
                if proj_pending:
                    emit_proj(proj_pending.pop(0))
                    n_op = 1
                else:
                    n_op = 3
                for _ in range(min(n_op, len(op_pending))):
                    emit_oproj(*op_pending.pop(0))
            for done in ctx_queue:
                emit_att_ctx(*done)
                if done[2] == QH_PER_CORE - 1:
                    op_pending.extend((done[0], done[1], qq) for qq in range(8))
            while op_pending:
                emit_oproj(*op_pending.pop(0), tail=True)
    nc.compile()
    return nc


def kernel(hidden_states, attention_mask, Wq, bq, Wk, bk, Wv, bv, Wo, bo):
    hidden_states = np.asarray(hidden_states, dtype=np.float32)
    attention_mask = np.asarray(attention_mask, dtype=np.float32)
    Wq = np.asarray(Wq, dtype=np.float32)
    Wk = np.asarray(Wk, dtype=np.float32)
    Wv = np.asarray(Wv, dtype=np.float32)
    Wo = np.asarray(Wo, dtype=np.float32)

    if "nc" not in _CACHE:
        _CACHE["nc"] = _build_program()
    nc = _CACHE["nc"]

    hsT = np.ascontiguousarray(
        hidden_states.reshape(TOK, H).T).astype(ml_dtypes.bfloat16)  # [H, B*S]
    maskp = np.ascontiguousarray(
        attention_mask.reshape(B, S // 128, 128).transpose(2, 0, 1))  # [128, B, 16]
    sb16 = A16 * (maskp + EXP_SHIFT) + B16CONST
    prep = np.stack([maskp + EXP_SHIFT, sb16, -sb16], axis=-1).astype(np.float32)
    prep = np.ascontiguousarray(prep)                     # [128, B, 16, 3]
    eye = np.eye(128, dtype=np.float32).astype(ml_dtypes.bfloat16)

    in_maps = []
    for c in range(N_CORES):
        wq = Wq[QF * c:QF * (c + 1)]          # [256, H]
        wk = Wk[D * c:D * (c + 1)]            # [64, H]
        wv = Wv[D * c:D * (c + 1)]            # [64, H]
        wqkvT = np.ascontiguousarray(
            np.concatenate([wq, wk, wv], axis=0).T).astype(ml_dtypes.bfloat16)
        woT = np.ascontiguousarray(
            Wo[:, QF * c:QF * (c + 1)].T).astype(ml_dtypes.bfloat16)          # [256, H]
        bq_c = bq[QF * c:QF * (c + 1)] * ALPHA
        bk_c = bk[D * c:D * (c + 1)] * BETA
        bv_c = bv[D * c:D * (c + 1)]
        bqkv_c = np.ascontiguousarray(
            np.concatenate([bq_c, bk_c, bv_c]).astype(np.float32)
            .reshape(3, 128).T)               # [128, 3]
        in_maps.append({
            "hsT": hsT, "wqkvT": wqkvT, "woT": woT,
            "bqkv": bqkv_c, "prep": prep, "eye": eye,
        })

    _CACHE["last_in_maps"] = in_maps
    res = bass_utils.run_bass_kernel_spmd(nc, in_maps, core_ids=list(range(N_CORES)))
    acc = np.zeros((B, S, H), dtype=np.float32)
    for c in range(N_CORES):
        acc += np.asarray(res.results[c]["out"], dtype=np.float32)
    acc += np.asarray(bo, dtype=np.float32)[None, None, :]
    return acc


# revision 43
# speedup vs baseline: 1.4985x; 1.0051x over previous
"""GQA attention kernel for Trainium2, sharded over 8 NeuronCores.

Sharding: tensor-parallel over heads. Core c owns kv-head c and q-heads
4c..4c+3 (rows 256c:256c+256 of Wq, rows 64c:64c+64 of Wk/Wv) and columns
256c:256c+256 of Wo. Each core computes a full-shape partial of the output
(o_proj column-parallel); the host sums the 8 partials (the all-reduce)
and adds bo.

Per-core kernel layout choices:
- hidden_states is passed transposed [H, B*S] in bf16 so QKV projections
  contract over the partition dim with one big contiguous DMA per 256-token
  chunk (descriptor-generation time is per-DMA).
- Q,K,V are produced transposed ([feature, token]) by the PE in bf16.
  Q,K carry a scale ALPHA/BETA folded into their PSUM drain so the scores
  PSUM lands directly in Schraudolph-exp units (see below).
- Scores are computed transposed, S^T[t, q] = K_d,t^T . Q_d,q, so the
  softmax mask/shift folds into the exp bias (per-partition), and a
  ones-column appended to V yields softmax denominators as row 64 of the
  context matmul output.
- The softmax exp is engine-split: ACT runs the Exp activation (bf16 out)
  for 12/16 kv tiles; DVE computes the other 4/16 with one tensor_scalar
  op via the Schraudolph bit trick targeted at bf16:
  bits16(exp(z)) ~= (128/ln2) z + 16249.6, and ALPHA*BETA is chosen so the
  scores PSUM already holds (128/ln2) * logit. out_i16 = max(psum, -b) + b
  with b = (128/ln2)(mask-2) + 16249.6; the -2 shift cancels in the
  normalize.
- Context is accumulated TRANSPOSED: ctx[q, d] tiles [128, 65] with the
  just-computed ex tile as the PE stationary operand and V(+ones) as the
  64+1-column moving operand — 65 charged rows per matmul instead of 512
  (the cost model charges moving rows only), halving context PE time.
  The denominator lands in column 64 (per-partition), so normalization is
  a reciprocal plus one per-partition tensor_scalar multiply on DVE, and
  the normalized tile is PE-transposed back to feature-major for o_proj.
- Emission is software-pipelined: after a startup phase that projects the
  first batch's tokens, the remaining projection chunks, V-transposes and
  all o_proj blocks are interleaved between attention g-blocks so the PE
  (the bottleneck engine) never drains. Projection/o_proj/transpose PSUM
  tiles share the two "ops" PSUM bank slots; tiny [1,1] wait-carrier
  matmuls pre-spend slot-reuse semaphore waits so every real PE
  instruction needs at most its one producer wait.
- o_proj uses f32r matmuls (full rate at N=512); output is drained to
  bf16 by DVE and DMA'd at half traffic; the host sums partials in fp32.
"""

import os
import sys

for _p in ("/opt/trn_rl_repo",):
    if _p not in sys.path and os.path.isdir(_p):
        sys.path.insert(0, _p)

import numpy as np
import ml_dtypes

import concourse.bass as bass
import concourse.bacc as bacc
import concourse.tile as tile
from concourse import mybir
from concourse import bass_utils

F32 = mybir.dt.float32
F32R = mybir.dt.float32r
BF16 = mybir.dt.bfloat16
I16 = mybir.dt.int16
AF = mybir.ActivationFunctionType

B = 2
S = 2048
H = 2048
D = 64
N_CORES = 8
QH_PER_CORE = 4          # q-heads per core
QF = QH_PER_CORE * D     # 256 q features per core
TOK = B * S              # 4096
SCALE = 1.0 / np.sqrt(D)  # 0.125
CK = 256                 # projection chunk tokens

# bf16 Schraudolph-exp calibration. scores_psum = ALPHA*BETA*(q.k) = A16*s
# where s = SCALE*(q.k) is the true logit and A16 = 128/ln2 is the bf16
# bits-per-logit slope.
A16 = 128.0 / np.log(2.0)                # 184.664
AB = A16 * SCALE                         # required ALPHA*BETA
ALPHA = float(np.sqrt(AB))               # q drain scale (4.804)
BETA = float(np.sqrt(AB))                # k drain scale
EXP_SHIFT = -2.0                         # exp(s + mask - 2) range centering
C16 = 0.058 * 128.0                      # Schraudolph mean-shift
B16CONST = 127.0 * 128.0 - C16 + 0.5     # +0.5: astype(int16) truncates
ACT_SCALE = float(1.0 / A16)             # ACT-path exp input scale

_CACHE = {}


def _exp_on_dve(b, qh, g, t):
    """Which kv tiles the DVE (vs ACT) computes exp for: 4/16 so ACT:DVE
    engine busy stays balanced against their other work. The first two
    blocks lean on DVE while ACT clears the projection-drain backlog."""
    if b == 0 and qh == 0 and g < 2:
        return t % 2 == 1
    return t % 4 == 3


def _build_program():
    nc = bacc.Bacc("TRN2", target_bir_lowering=False, debug=False)

    hsT = nc.dram_tensor("hsT", [H, TOK], BF16, kind="ExternalInput").ap()
    wqkvT = nc.dram_tensor("wqkvT", [H, 384], BF16, kind="ExternalInput").ap()
    woT = nc.dram_tensor("woT", [QF, H], BF16, kind="ExternalInput").ap()
    bqkv = nc.dram_tensor("bqkv", [128, 3], F32, kind="ExternalInput").ap()
    prep = nc.dram_tensor("prep", [128, B, S // 128, 3], F32, kind="ExternalInput").ap()
    eye = nc.dram_tensor("eye", [128, 128], BF16, kind="ExternalInput").ap()
    out = nc.dram_tensor("out", [B, S, H], BF16, kind="ExternalOutput").ap()

    with tile.TileContext(nc) as tc:
        with tc.tile_pool(name="const", bufs=1) as cp, \
             tc.tile_pool(name="proj_sb", bufs=5) as psb, \
             tc.tile_pool(name="exb_sb", bufs=34) as ebp, \
             tc.tile_pool(name="drain_sb", bufs=3) as dsb, \
             tc.tile_pool(name="osb_sb", bufs=3) as osp, \
             tc.tile_pool(name="ctxT_sb", bufs=3) as csb, \
             tc.tile_pool(name="scores_ps", bufs=2, space="PSUM") as sps, \
             tc.tile_pool(name="ctx_ps", bufs=2, space="PSUM") as xps, \
             tc.tile_pool(name="o_ps", bufs=2, space="PSUM") as ops_pool:
            w_qkv = cp.tile([128, 16, 384], BF16)     # (p, h_tile, feature)
            wq_t = wqkvT.rearrange("(t p) f -> p t f", p=128)
            nc.sync.dma_start(out=w_qkv[:, 0:8, :], in_=wq_t[:, 0:8, :])
            nc.sync.dma_start(out=w_qkv[:, 8:16, :], in_=wq_t[:, 8:16, :])
            bqkv_sb = cp.tile([128, 3], F32)
            nc.sync.dma_start(out=bqkv_sb, in_=bqkv)
            eye_sb = cp.tile([128, 128], BF16)
            nc.sync.dma_start(out=eye_sb, in_=eye)
            # w_o and prep are first consumed tens of microseconds in; issue
            # their DMAs behind the first projection chunk's token loads.
            w_o = cp.tile([128, 2, H], BF16)          # (p, f_tile, e)
            prep_sb = cp.tile([128, B, S // 128, 3], F32)

            def emit_late_consts():
                nc.sync.dma_start(out=w_o, in_=woT.rearrange("(t p) e -> p t e", p=128))
                nc.sync.dma_start(out=prep_sb, in_=prep)

            # Engine wait budgets are tiny (1 sync-wait per instruction for
            # PE/ACT structs). Warm consumer-engine vector clocks on the
            # small const DMAs so real instructions never need extra waits.
            scratch = cp.tile([128, 1], F32)
            nc.scalar.copy(out=scratch, in_=bqkv_sb[:, 0:1])
            nc.scalar.copy(out=scratch, in_=prep_sb[:, 0, 0, 0:1])
            scratch_d = cp.tile([128, 1], F32)
            nc.vector.tensor_copy(out=scratch_d, in_=prep_sb[:, 0, 0, 1:2])

            # Q^T bf16 scaled by ALPHA (2 q-heads per tile), V^T bf16
            # (partitions 64:128), K^T bf16 scaled by BETA replicated in
            # both partition halves (PE base-partition alignment for the
            # per-head scores matmuls).
            qT = [cp.tile([128, TOK], BF16, name=f"qT{i}") for i in range(2)]
            vT = cp.tile([128, TOK], BF16)
            k2 = cp.tile([128, TOK], BF16)
            # V transposed back to [t, d] + ones column, per 128-token tile.
            vones = cp.tile([128, B * 16, 65], BF16)

            hsT_tiled = hsT.rearrange("(t p) n -> p t n", p=128)

            def dummy_mm(ps_tile, reader):
                """[1,1] wait-carrier matmul: first touch of a PSUM slot,
                spending its slot-reuse wait; `reader` must be an SBUF AP
                whose producer clock the PE already knows."""
                nc.tensor.matmul(ps_tile[0:1, 0:1], reader, reader,
                                 start=True, stop=True)

            w_r = w_qkv[:, 0, 0:2].bitcast(F32)[:, 0:1]
            wo_r = w_o[:, 0, 0:2].bitcast(F32)[:, 0:1]
            eye_r = eye_sb[0:64, 0:2].bitcast(F32)[:, 0:1]

            stage_map = {}

            def emit_proj_dma(ck):
                hstage = psb.tile([128, 16, CK], BF16, tag="hstage", name=f"hs_{ck}")
                nc.sync.dma_start(
                    out=hstage, in_=hsT_tiled[:, :, ck * CK:(ck + 1) * CK])
                stage_map[ck] = hstage

            def emit_proj(ck, startup=False):
                if ck not in stage_map:
                    emit_proj_dma(ck)
                hstage = stage_map.pop(ck)
                cols = slice(ck * CK, (ck + 1) * CK)
                for ft in range(3):
                    # during startup the attention ctx PSUM slots are idle;
                    # alternating pools doubles the slot-recycle distance
                    pool = xps if startup and ft == 1 else ops_pool
                    ps = pool.tile([128, 512], F32, tag="ctx" if pool is xps else "ops",
                                   name=f"pj_{ck}_{ft}")
                    # carrier takes the slot-release wait; the first real
                    # matmul then only waits on the hstage DMA.
                    dummy_mm(ps, w_r)
                    for ht in range(16):
                        nc.tensor.matmul(
                            ps[:, 0:CK],
                            w_qkv[:, ht, ft * 128:(ft + 1) * 128],
                            hstage[:, ht, :],
                            start=(ht == 0), stop=(ht == 15),
                        )
                    if ft < 2:
                        nc.scalar.activation(
                            out=qT[ft][:, cols], in_=ps[:, 0:CK],
                            func=AF.Identity, bias=bqkv_sb[:, ft:ft + 1],
                            scale=ALPHA,
                        )
                    else:
                        nc.scalar.activation(
                            out=k2[0:64, cols], in_=ps[0:64, 0:CK],
                            func=AF.Identity, bias=bqkv_sb[0:64, 2:3],
                            scale=BETA,
                        )
                        # replicate this chunk's K into the upper partition
                        # half right away (PE base-partition alignment for
                        # qp=64 heads) instead of one big late DMA
                        nc.sync.dma_start(out=k2[64:128, cols], in_=k2[0:64, cols])
                        nc.scalar.activation(
                            out=vT[64:128, cols], in_=ps[64:128, 0:CK],
                            func=AF.Identity, bias=bqkv_sb[64:128, 2:3],
                        )
                # V[t, d] tiles for the two 128-token tiles of this chunk
                # (both share one ops slot at different bf16 column ranges)
                pool = xps if startup else ops_pool
                tp = pool.tile([128, 512], F32, tag="ctx" if startup else "ops",
                               name=f"tp_{ck}")
                dummy_mm(tp, eye_r)
                for i, bt in enumerate((2 * ck, 2 * ck + 1)):
                    tpb = tp.bitcast(BF16)[:, 64 * i:64 * i + 64]
                    nc.tensor.transpose(
                        tpb, in_=vT[64:128, bt * 128:(bt + 1) * 128],
                        identity=eye_sb[64:128, 64:128])
                    nc.scalar.copy(out=vones[:, bt, 0:64], in_=tpb)

            def emit_k2_repl(b):
                # ones column for this batch's vones tiles (ACT: scale 0)
                nc.scalar.activation(
                    out=vones[:, b * 16:(b + 1) * 16, 64:65],
                    in_=prep_sb[:, b, :, 0],
                    func=AF.Identity, bias=1.0, scale=0.0)
                # spend the k2-replication DMA wait on the PE clock
                dps = sps.tile([128, 1024], F32, tag="scores", name=f"k2d_{b}")
                nc.tensor.matmul(dps[0:1, 0:1],
                                 k2[64:128, b * S:b * S + 2].bitcast(F32)[:, 0:1],
                                 k2[64:128, b * S:b * S + 2].bitcast(F32)[:, 0:1],
                                 start=True, stop=True)

            ctxT_tiles = {}
            exs_map = {}

            def emit_att_sc(b, qh, g):
                q0 = b * S + qh * 1024
                if g == 0:
                    # No slot pre-spend needed: the first writer (the DVE
                    # tensor_copy) and the slot releaser (PE o_proj reads)
                    # pair with the PE-side transpose producer wait.
                    ctxT = [csb.tile([128, 1024], BF16, tag=f"ctxT{ft}",
                                     name=f"ctxT{ft}_{b}_{qh}") for ft in range(2)]
                    ctxT_tiles[(b, qh)] = ctxT
                ctxT = ctxT_tiles[(b, qh)]
                qt = qT[g // 2]
                qp = (g % 2) * 64
                exs = []
                for t in range(16):
                    sc = sps.tile([128, 1024], F32, tag="scores",
                                  name=f"sc_{b}_{qh}_{g}_{t}")
                    for qc in range(2):
                        nc.tensor.matmul(
                            sc[:, qc * 512:(qc + 1) * 512],
                            k2[qp:qp + 64, b * S + t * 128:b * S + (t + 1) * 128],
                            qt[qp:qp + 64, q0 + qc * 512:q0 + (qc + 1) * 512],
                            start=True, stop=True,
                        )
                    ex = ebp.tile([128, 1024], BF16, tag="expT",
                                  name=f"ex_{b}_{qh}_{g}_{t}")
                    exs.append(ex)
                    if _exp_on_dve(b, qh, g, t):
                        nc.vector.tensor_scalar(
                            out=ex.bitcast(I16), in0=sc,
                            scalar1=prep_sb[:, b, t, 2:3],
                            scalar2=prep_sb[:, b, t, 1:2],
                            op0=mybir.AluOpType.max,
                            op1=mybir.AluOpType.add,
                        )
                    else:
                        nc.scalar.activation(
                            out=ex, in_=sc, func=AF.Exp,
                            bias=prep_sb[:, b, t, 0:1],
                            scale=ACT_SCALE,
                        )
                exs_map[(b, qh, g)] = exs

            def emit_att_ctx(b, qh, g):
                # transposed context: per 128-token q subtile, accumulate
                # ctx[q, d|den] over all kv tiles with ex as the stationary
                # operand (65 charged rows per matmul), then normalize by
                # the per-partition denominator column and transpose back.
                # Emitted one block late so the PE never head-of-line waits
                # on this block's own exp results.
                ctxT = ctxT_tiles[(b, qh)]
                exs = exs_map.pop((b, qh, g))
                qp = (g % 2) * 64
                for qp2 in range(4):        # pairs of 128-token q subtiles
                    cn = dsb.tile([128, 128], BF16, tag="ctxn",
                                  name=f"cn_{b}_{qh}_{g}_{qp2}")
                    for i in range(2):
                        qs = 2 * qp2 + i
                        cx = xps.tile([128, 65], F32, tag="ctx",
                                      name=f"cx_{b}_{qh}_{g}_{qs}")
                        # wait-carrier: spend the ctx-slot WAR wait (DVE
                        # release) before the real t=0 accumulation start.
                        dummy_mm(cx, wo_r)
                        for t in range(16):
                            nc.tensor.matmul(
                                cx,
                                exs[t][:, qs * 128:(qs + 1) * 128],
                                vones[:, b * 16 + t, :],
                                start=(t == 0), stop=(t == 15),
                            )
                        rc = dsb.tile([128, 1], F32, tag="recip",
                                      name=f"rc_{b}_{qh}_{g}_{qs}")
                        nc.vector.reciprocal(out=rc, in_=cx[:, 64:65])
                        nc.vector.tensor_scalar(
                            out=cn[:, 64 * i:64 * i + 64], in0=cx[:, 0:64],
                            scalar1=rc, scalar2=None,
                            op0=mybir.AluOpType.mult,
                        )
                    # one transpose covers both subtiles: out rows 0:64 are
                    # subtile 2*qp2 features, rows 64:128 the other's
                    tq = ops_pool.tile([128, 512], F32, tag="ops",
                                       name=f"tq_{b}_{qh}_{g}_{qp2}")
                    dummy_mm(tq, eye_r)
                    tqb = tq.bitcast(BF16)[:, 0:128]
                    nc.tensor.transpose(tqb, in_=cn, identity=eye_sb)
                    for i in range(2):
                        qs = 2 * qp2 + i
                        nc.vector.tensor_copy(
                            out=ctxT[g // 2][qp:qp + 64, qs * 128:(qs + 1) * 128],
                            in_=tqb[64 * i:64 * i + 64, :],
                        )

            def emit_oproj(b, qh, qq, tail=False):
                ctxT = ctxT_tiles[(b, qh)]
                osb = osp.tile([128, H], BF16, tag="osb", name=f"osb_{b}_{qh}_{qq}")
                # pre-spend the osb slot-reuse wait (out-DMA done)
                nc.vector.memset(osb[0:1, 0:1], 0.0)
                rows = slice(qh * 1024 + qq * 128, qh * 1024 + (qq + 1) * 128)
                for ec in range(4):
                    op = ops_pool.tile([128, 512], F32, tag="ops",
                                       name=f"op_{b}_{qh}_{qq}_{ec}")
                    # carrier takes the slot-release wait (mixed ACT/DVE
                    # releasers from interleaved proj drains)
                    dummy_mm(op, wo_r)
                    for ft in range(2):
                        nc.tensor.matmul(
                            op,
                            ctxT[ft][:, qq * 128:(qq + 1) * 128],
                            w_o[:, ft, ec * 512:(ec + 1) * 512],
                            start=(ft == 0), stop=(ft == 1),
                        )
                    if tail and ec % 2 == 1:
                        nc.scalar.copy(
                            out=osb[:, ec * 512:(ec + 1) * 512], in_=op)
                    else:
                        nc.vector.tensor_copy(
                            out=osb[:, ec * 512:(ec + 1) * 512], in_=op)
                    if tail and ec == 1:
                        # start the first half of the output write early so
                        # the end-of-program DMA drain is shorter
                        nc.sync.dma_start(out=out[b, rows, 0:1024],
                                          in_=osb[:, 0:1024])
                if tail:
                    nc.sync.dma_start(out=out[b, rows, 1024:2048],
                                      in_=osb[:, 1024:2048])
                else:
                    nc.sync.dma_start(out=out[b, rows, :], in_=osb)

            # ---- software-pipelined emission schedule ----
            for ck in range(4):
                emit_proj_dma(ck)
            emit_proj(0, startup=True)
            emit_late_consts()
            for ck in range(1, 8):
                if ck + 3 < 8:
                    emit_proj_dma(ck + 3)
                emit_proj(ck, startup=True)
            emit_k2_repl(0)

            proj_pending = list(range(8, 16))
            op_pending = []
            att_blocks = [(b, qh, g) for b in range(B) for qh in range(2)
                          for g in range(QH_PER_CORE)]
            ctx_queue = []
            for i, (b, qh, g) in enumerate(att_blocks):
                if proj_pending and proj_pending[0] not in stage_map:
                    emit_proj_dma(proj_pending[0])
                emit_att_sc(b, qh, g)
                ctx_queue.append((b, qh, g))
                if len(ctx_queue) > 2:
                    done = ctx_queue.pop(0)
                    emit_att_ctx(*done)
                    if done[2] == QH_PER_CORE - 1:
                        op_pending.extend((done[0], done[1], qq) for qq in range(8))
                if i == 7:
                    # b=1 attention needs all projections + its K replica
                    while proj_pending:
                        emit_proj(proj_pending.pop(0))
                    emit_k2_repl(1)
                    continue
                if proj_pending:
                    emit_proj(proj_pending.pop(0))
                    n_op = 1
                else:
                    n_op = 3
                for _ in range(min(n_op, len(op_pending))):
                    emit_oproj(*op_pending.pop(0))
            for done in ctx_queue:
                emit_att_ctx(*done)
                if done[2] == QH_PER_CORE - 1:
                    op_pending.extend((done[0], done[1], qq) for qq in range(8))
            while op_pending:
                emit_oproj(*op_pending.pop(0), tail=True)
    nc.compile()
    return nc


def kernel(hidden_states, attention_mask, Wq, bq, Wk, bk, Wv, bv, Wo, bo):
    hidden_states = np.asarray(hidden_states, dtype=np.float32)
    attention_mask = np.asarray(attention_mask, dtype=np.float32)
    Wq = np.asarray(Wq, dtype=np.float32)
    Wk = np.asarray(Wk, dtype=np.float32)
    Wv = np.asarray(Wv, dtype=np.float32)
    Wo = np.asarray(Wo, dtype=np.float32)

    if "nc" not in _CACHE:
        _CACHE["nc"] = _build_program()
    nc = _CACHE["nc"]

    hsT = np.ascontiguousarray(
        hidden_states.reshape(TOK, H).T).astype(ml_dtypes.bfloat16)  # [H, B*S]
    maskp = np.ascontiguousarray(
        attention_mask.reshape(B, S // 128, 128).transpose(2, 0, 1))  # [128, B, 16]
    sb16 = A16 * (maskp + EXP_SHIFT) + B16CONST
    prep = np.stack([maskp + EXP_SHIFT, sb16, -sb16], axis=-1).astype(np.float32)
    prep = np.ascontiguousarray(prep)                     # [128, B, 16, 3]
    eye = np.eye(128, dtype=np.float32).astype(ml_dtypes.bfloat16)

    in_maps = []
    for c in range(N_CORES):
        wq = Wq[QF * c:QF * (c + 1)]          # [256, H]
        wk = Wk[D * c:D * (c + 1)]            # [64, H]
        wv = Wv[D * c:D * (c + 1)]            # [64, H]
        wqkvT = np.ascontiguousarray(
            np.concatenate([wq, wk, wv], axis=0).T).astype(ml_dtypes.bfloat16)
        woT = np.ascontiguousarray(
            Wo[:, QF * c:QF * (c + 1)].T).astype(ml_dtypes.bfloat16)          # [256, H]
        bq_c = bq[QF * c:QF * (c + 1)] * ALPHA
        bk_c = bk[D * c:D * (c + 1)] * BETA
        bv_c = bv[D * c:D * (c + 1)]
        bqkv_c = np.ascontiguousarray(
            np.concatenate([bq_c, bk_c, bv_c]).astype(np.float32)
            .reshape(3, 128).T)               # [128, 3]
        in_maps.append({
            "hsT": hsT, "wqkvT": wqkvT, "woT": woT,
            "bqkv": bqkv_c, "prep": prep, "eye": eye,
        })

    _CACHE["last_in_maps"] = in_maps
    res = bass_utils.run_bass_kernel_spmd(nc, in_maps, core_ids=list(range(N_CORES)))
    acc = np.zeros((B, S, H), dtype=np.float32)
    for c in range(N_CORES):
        acc += np.asarray(res.results[c]["out"], dtype=np.float32)
    acc += np.asarray(bo, dtype=np.float32)[None, None, :]
    return acc


# revision 44
# speedup vs baseline: 1.5127x; 1.0095x over previous
"""GQA attention kernel for Trainium2, sharded over 8 NeuronCores.

Sharding: tensor-parallel over heads. Core c owns kv-head c and q-heads
4c..4c+3 (rows 256c:256c+256 of Wq, rows 64c:64c+64 of Wk/Wv) and columns
256c:256c+256 of Wo. Each core computes a full-shape partial of the output
(o_proj column-parallel); the host sums the 8 partials (the all-reduce)
and adds bo.

Per-core kernel layout choices:
- hidden_states is passed transposed [H, B*S] in bf16 so QKV projections
  contract over the partition dim with one big contiguous DMA per 256-token
  chunk (descriptor-generation time is per-DMA).
- Q,K,V are produced transposed ([feature, token]) by the PE in bf16.
  Q,K carry a scale ALPHA/BETA folded into their PSUM drain so the scores
  PSUM lands directly in Schraudolph-exp units (see below).
- Scores are computed transposed, S^T[t, q] = K_d,t^T . Q_d,q, so the
  softmax mask/shift folds into the exp bias (per-partition), and a
  ones-column appended to V yields softmax denominators as row 64 of the
  context matmul output.
- The softmax exp is engine-split: ACT runs the Exp activation (bf16 out)
  for 12/16 kv tiles; DVE computes the other 4/16 with one tensor_scalar
  op via the Schraudolph bit trick targeted at bf16:
  bits16(exp(z)) ~= (128/ln2) z + 16249.6, and ALPHA*BETA is chosen so the
  scores PSUM already holds (128/ln2) * logit. out_i16 = max(psum, -b) + b
  with b = (128/ln2)(mask-2) + 16249.6; the -2 shift cancels in the
  normalize.
- Context is accumulated TRANSPOSED: ctx[q, d] tiles [128, 65] with the
  just-computed ex tile as the PE stationary operand and V(+ones) as the
  64+1-column moving operand — 65 charged rows per matmul instead of 512
  (the cost model charges moving rows only), halving context PE time.
  The denominator lands in column 64 (per-partition), so normalization is
  a reciprocal plus one per-partition tensor_scalar multiply on DVE, and
  the normalized tile is PE-transposed back to feature-major for o_proj.
- Emission is software-pipelined: after a startup phase that projects the
  first batch's tokens, the remaining projection chunks, V-transposes and
  all o_proj blocks are interleaved between attention g-blocks so the PE
  (the bottleneck engine) never drains. Projection/o_proj/transpose PSUM
  tiles share the two "ops" PSUM bank slots; tiny [1,1] wait-carrier
  matmuls pre-spend slot-reuse semaphore waits so every real PE
  instruction needs at most its one producer wait.
- o_proj uses bf16 matmuls; output is drained to bf16 by DVE (plus ACT in
  the tail) and DMA'd at half traffic; the host sums partials in fp32.
- Projection-chunk DMAs are prefetched ahead of their compute and K's
  upper-half replica is copied incrementally per chunk, hiding the
  ~5.3us end-to-end DMA latency (HWDGE + DGE delay + transfer + sem).
"""

import os
import sys

for _p in ("/opt/trn_rl_repo",):
    if _p not in sys.path and os.path.isdir(_p):
        sys.path.insert(0, _p)

import numpy as np
import ml_dtypes

import concourse.bass as bass
import concourse.bacc as bacc
import concourse.tile as tile
from concourse import mybir
from concourse import bass_utils

F32 = mybir.dt.float32
F32R = mybir.dt.float32r
BF16 = mybir.dt.bfloat16
I16 = mybir.dt.int16
AF = mybir.ActivationFunctionType

B = 2
S = 2048
H = 2048
D = 64
N_CORES = 8
QH_PER_CORE = 4          # q-heads per core
QF = QH_PER_CORE * D     # 256 q features per core
TOK = B * S              # 4096
SCALE = 1.0 / np.sqrt(D)  # 0.125
CK = 256                 # projection chunk tokens

# bf16 Schraudolph-exp calibration. scores_psum = ALPHA*BETA*(q.k) = A16*s
# where s = SCALE*(q.k) is the true logit and A16 = 128/ln2 is the bf16
# bits-per-logit slope.
A16 = 128.0 / np.log(2.0)                # 184.664
AB = A16 * SCALE                         # required ALPHA*BETA
ALPHA = float(np.sqrt(AB))               # q drain scale (4.804)
BETA = float(np.sqrt(AB))                # k drain scale
EXP_SHIFT = -2.0                         # exp(s + mask - 2) range centering
C16 = 0.058 * 128.0                      # Schraudolph mean-shift
B16CONST = 127.0 * 128.0 - C16 + 0.5     # +0.5: astype(int16) truncates
ACT_SCALE = float(1.0 / A16)             # ACT-path exp input scale

_CACHE = {}


def _exp_on_dve(b, qh, g, t):
    """Which kv tiles the DVE (vs ACT) computes exp for: 4/16 so ACT:DVE
    engine busy stays balanced against their other work. The first two
    blocks lean on DVE while ACT clears the projection-drain backlog."""
    if b == 0 and qh == 0 and g < 2:
        return t % 2 == 1
    return t % 4 == 3


def _build_program():
    nc = bacc.Bacc("TRN2", target_bir_lowering=False, debug=False)

    hsT = nc.dram_tensor("hsT", [H, TOK], BF16, kind="ExternalInput").ap()
    wqkvT = nc.dram_tensor("wqkvT", [H, 384], BF16, kind="ExternalInput").ap()
    woT = nc.dram_tensor("woT", [QF, H], BF16, kind="ExternalInput").ap()
    bqkv = nc.dram_tensor("bqkv", [128, 3], F32, kind="ExternalInput").ap()
    prep = nc.dram_tensor("prep", [128, B, S // 128, 3], F32, kind="ExternalInput").ap()
    eye = nc.dram_tensor("eye", [128, 128], BF16, kind="ExternalInput").ap()
    out = nc.dram_tensor("out", [B, S, H], BF16, kind="ExternalOutput").ap()

    with tile.TileContext(nc) as tc:
        with tc.tile_pool(name="const", bufs=1) as cp, \
             tc.tile_pool(name="proj_sb", bufs=5) as psb, \
             tc.tile_pool(name="exb_sb", bufs=34) as ebp, \
             tc.tile_pool(name="drain_sb", bufs=3) as dsb, \
             tc.tile_pool(name="osb_sb", bufs=3) as osp, \
             tc.tile_pool(name="ctxT_sb", bufs=3) as csb, \
             tc.tile_pool(name="scores_ps", bufs=2, space="PSUM") as sps, \
             tc.tile_pool(name="ctx_ps", bufs=2, space="PSUM") as xps, \
             tc.tile_pool(name="o_ps", bufs=2, space="PSUM") as ops_pool:
            w_qkv = cp.tile([128, 16, 384], BF16)     # (p, h_tile, feature)
            wq_t = wqkvT.rearrange("(t p) f -> p t f", p=128)
            nc.sync.dma_start(out=w_qkv[:, 0:8, :], in_=wq_t[:, 0:8, :])
            nc.sync.dma_start(out=w_qkv[:, 8:16, :], in_=wq_t[:, 8:16, :])
            bqkv_sb = cp.tile([128, 3], F32)
            nc.sync.dma_start(out=bqkv_sb, in_=bqkv)
            eye_sb = cp.tile([128, 128], BF16)
            nc.sync.dma_start(out=eye_sb, in_=eye)
            # w_o and prep are first consumed tens of microseconds in; issue
            # their DMAs behind the first projection chunk's token loads.
            w_o = cp.tile([128, 2, H], BF16)          # (p, f_tile, e)
            prep_sb = cp.tile([128, B, S // 128, 3], F32)

            def emit_late_consts():
                nc.sync.dma_start(out=w_o, in_=woT.rearrange("(t p) e -> p t e", p=128))
                nc.sync.dma_start(out=prep_sb, in_=prep)

            # Engine wait budgets are tiny (1 sync-wait per instruction for
            # PE/ACT structs). Warm consumer-engine vector clocks on the
            # small const DMAs so real instructions never need extra waits.
            scratch = cp.tile([128, 1], F32)
            nc.scalar.copy(out=scratch, in_=bqkv_sb[:, 0:1])
            nc.scalar.copy(out=scratch, in_=prep_sb[:, 0, 0, 0:1])
            scratch_d = cp.tile([128, 1], F32)
            nc.vector.tensor_copy(out=scratch_d, in_=prep_sb[:, 0, 0, 1:2])

            # Q^T bf16 scaled by ALPHA (2 q-heads per tile), V^T bf16
            # (partitions 64:128), K^T bf16 scaled by BETA replicated in
            # both partition halves (PE base-partition alignment for the
            # per-head scores matmuls).
            qT = [cp.tile([128, TOK], BF16, name=f"qT{i}") for i in range(2)]
            vT = cp.tile([128, TOK], BF16)
            k2 = cp.tile([128, TOK], BF16)
            # V transposed back to [t, d] + ones column, per 128-token tile.
            vones = cp.tile([128, B * 16, 65], BF16)

            hsT_tiled = hsT.rearrange("(t p) n -> p t n", p=128)

            def dummy_mm(ps_tile, reader):
                """[1,1] wait-carrier matmul: first touch of a PSUM slot,
                spending its slot-reuse wait; `reader` must be an SBUF AP
                whose producer clock the PE already knows."""
                nc.tensor.matmul(ps_tile[0:1, 0:1], reader, reader,
                                 start=True, stop=True)

            w_r = w_qkv[:, 0, 0:2].bitcast(F32)[:, 0:1]
            wo_r = w_o[:, 0, 0:2].bitcast(F32)[:, 0:1]
            eye_r = eye_sb[0:64, 0:2].bitcast(F32)[:, 0:1]

            stage_map = {}

            def emit_proj_dma(ck):
                hstage = psb.tile([128, 16, CK], BF16, tag="hstage", name=f"hs_{ck}")
                nc.sync.dma_start(
                    out=hstage, in_=hsT_tiled[:, :, ck * CK:(ck + 1) * CK])
                stage_map[ck] = hstage

            def emit_proj(ck, startup=False):
                if ck not in stage_map:
                    emit_proj_dma(ck)
                hstage = stage_map.pop(ck)
                cols = slice(ck * CK, (ck + 1) * CK)
                for ft in range(3):
                    # during startup the attention ctx PSUM slots are idle;
                    # alternating pools doubles the slot-recycle distance
                    pool = xps if startup and ft == 1 else ops_pool
                    ps = pool.tile([128, 512], F32, tag="ctx" if pool is xps else "ops",
                                   name=f"pj_{ck}_{ft}")
                    # carrier takes the slot-release wait; the first real
                    # matmul then only waits on the hstage DMA.
                    dummy_mm(ps, w_r)
                    for ht in range(16):
                        nc.tensor.matmul(
                            ps[:, 0:CK],
                            w_qkv[:, ht, ft * 128:(ft + 1) * 128],
                            hstage[:, ht, :],
                            start=(ht == 0), stop=(ht == 15),
                        )
                    if ft < 2:
                        nc.scalar.activation(
                            out=qT[ft][:, cols], in_=ps[:, 0:CK],
                            func=AF.Identity, bias=bqkv_sb[:, ft:ft + 1],
                            scale=ALPHA,
                        )
                    else:
                        nc.scalar.activation(
                            out=k2[0:64, cols], in_=ps[0:64, 0:CK],
                            func=AF.Identity, bias=bqkv_sb[0:64, 2:3],
                            scale=BETA,
                        )
                        # replicate this chunk's K into the upper partition
                        # half right away (PE base-partition alignment for
                        # qp=64 heads) instead of one big late DMA
                        nc.sync.dma_start(out=k2[64:128, cols], in_=k2[0:64, cols])
                        nc.scalar.activation(
                            out=vT[64:128, cols], in_=ps[64:128, 0:CK],
                            func=AF.Identity, bias=bqkv_sb[64:128, 2:3],
                        )
                # V[t, d] tiles for the two 128-token tiles of this chunk
                # (both share one ops slot at different bf16 column ranges)
                pool = xps if startup else ops_pool
                tp = pool.tile([128, 512], F32, tag="ctx" if startup else "ops",
                               name=f"tp_{ck}")
                dummy_mm(tp, eye_r)
                for i, bt in enumerate((2 * ck, 2 * ck + 1)):
                    tpb = tp.bitcast(BF16)[:, 64 * i:64 * i + 64]
                    nc.tensor.transpose(
                        tpb, in_=vT[64:128, bt * 128:(bt + 1) * 128],
                        identity=eye_sb[64:128, 64:128])
                    nc.scalar.copy(out=vones[:, bt, 0:64], in_=tpb)

            def emit_k2_repl(b):
                # ones column for this batch's vones tiles (ACT: scale 0)
                nc.scalar.activation(
                    out=vones[:, b * 16:(b + 1) * 16, 64:65],
                    in_=prep_sb[:, b, :, 0],
                    func=AF.Identity, bias=1.0, scale=0.0)
                # spend the k2-replication DMA wait on the PE clock
                dps = sps.tile([128, 1024], F32, tag="scores", name=f"k2d_{b}")
                nc.tensor.matmul(dps[0:1, 0:1],
                                 k2[64:128, b * S:b * S + 2].bitcast(F32)[:, 0:1],
                                 k2[64:128, b * S:b * S + 2].bitcast(F32)[:, 0:1],
                                 start=True, stop=True)

            ctxT_tiles = {}
            exs_map = {}

            def emit_att_sc(b, qh, g):
                q0 = b * S + qh * 1024
                if g == 0:
                    # No slot pre-spend needed: the first writer (the DVE
                    # tensor_copy) and the slot releaser (PE o_proj reads)
                    # pair with the PE-side transpose producer wait.
                    ctxT = [csb.tile([128, 1024], BF16, tag=f"ctxT{ft}",
                                     name=f"ctxT{ft}_{b}_{qh}") for ft in range(2)]
                    ctxT_tiles[(b, qh)] = ctxT
                ctxT = ctxT_tiles[(b, qh)]
                qt = qT[g // 2]
                qp = (g % 2) * 64
                exs = []
                for t in range(16):
                    sc = sps.tile([128, 1024], F32, tag="scores",
                                  name=f"sc_{b}_{qh}_{g}_{t}")
                    for qc in range(2):
                        nc.tensor.matmul(
                            sc[:, qc * 512:(qc + 1) * 512],
                            k2[qp:qp + 64, b * S + t * 128:b * S + (t + 1) * 128],
                            qt[qp:qp + 64, q0 + qc * 512:q0 + (qc + 1) * 512],
                            start=True, stop=True,
                        )
                    ex = ebp.tile([128, 1024], BF16, tag="expT",
                                  name=f"ex_{b}_{qh}_{g}_{t}")
                    exs.append(ex)
                    if _exp_on_dve(b, qh, g, t):
                        nc.vector.tensor_scalar(
                            out=ex.bitcast(I16), in0=sc,
                            scalar1=prep_sb[:, b, t, 2:3],
                            scalar2=prep_sb[:, b, t, 1:2],
                            op0=mybir.AluOpType.max,
                            op1=mybir.AluOpType.add,
                        )
                    else:
                        nc.scalar.activation(
                            out=ex, in_=sc, func=AF.Exp,
                            bias=prep_sb[:, b, t, 0:1],
                            scale=ACT_SCALE,
                        )
                exs_map[(b, qh, g)] = exs

            def emit_att_ctx(b, qh, g):
                # transposed context: per 128-token q subtile, accumulate
                # ctx[q, d|den] over all kv tiles with ex as the stationary
                # operand (65 charged rows per matmul), then normalize by
                # the per-partition denominator column and transpose back.
                # Emitted one block late so the PE never head-of-line waits
                # on this block's own exp results.
                ctxT = ctxT_tiles[(b, qh)]
                exs = exs_map.pop((b, qh, g))
                qp = (g % 2) * 64
                for qp2 in range(4):        # pairs of 128-token q subtiles
                    cn = dsb.tile([128, 128], BF16, tag="ctxn",
                                  name=f"cn_{b}_{qh}_{g}_{qp2}")
                    for i in range(2):
                        qs = 2 * qp2 + i
                        cx = xps.tile([128, 65], F32, tag="ctx",
                                      name=f"cx_{b}_{qh}_{g}_{qs}")
                        # wait-carrier: spend the ctx-slot WAR wait (DVE
                        # release) before the real t=0 accumulation start.
                        dummy_mm(cx, wo_r)
                        for t in range(16):
                            nc.tensor.matmul(
                                cx,
                                exs[t][:, qs * 128:(qs + 1) * 128],
                                vones[:, b * 16 + t, :],
                                start=(t == 0), stop=(t == 15),
                            )
                        rc = dsb.tile([128, 1], F32, tag="recip",
                                      name=f"rc_{b}_{qh}_{g}_{qs}")
                        nc.vector.reciprocal(out=rc, in_=cx[:, 64:65])
                        nc.vector.tensor_scalar(
                            out=cn[:, 64 * i:64 * i + 64], in0=cx[:, 0:64],
                            scalar1=rc, scalar2=None,
                            op0=mybir.AluOpType.mult,
                        )
                    # one transpose covers both subtiles: out rows 0:64 are
                    # subtile 2*qp2 features, rows 64:128 the other's
                    tq = ops_pool.tile([128, 512], F32, tag="ops",
                                       name=f"tq_{b}_{qh}_{g}_{qp2}")
                    dummy_mm(tq, eye_r)
                    tqb = tq.bitcast(BF16)[:, 0:128]
                    nc.tensor.transpose(tqb, in_=cn, identity=eye_sb)
                    for i in range(2):
                        qs = 2 * qp2 + i
                        nc.vector.tensor_copy(
                            out=ctxT[g // 2][qp:qp + 64, qs * 128:(qs + 1) * 128],
                            in_=tqb[64 * i:64 * i + 64, :],
                        )

            def emit_oproj(b, qh, qq, tail=False):
                ctxT = ctxT_tiles[(b, qh)]
                osb = osp.tile([128, H], BF16, tag="osb", name=f"osb_{b}_{qh}_{qq}")
                # pre-spend the osb slot-reuse wait (out-DMA done)
                nc.vector.memset(osb[0:1, 0:1], 0.0)
                rows = slice(qh * 1024 + qq * 128, qh * 1024 + (qq + 1) * 128)
                for ec in range(4):
                    op = ops_pool.tile([128, 512], F32, tag="ops",
                                       name=f"op_{b}_{qh}_{qq}_{ec}")
                    # carrier takes the slot-release wait (mixed ACT/DVE
                    # releasers from interleaved proj drains)
                    dummy_mm(op, wo_r)
                    for ft in range(2):
                        nc.tensor.matmul(
                            op,
                            ctxT[ft][:, qq * 128:(qq + 1) * 128],
                            w_o[:, ft, ec * 512:(ec + 1) * 512],
                            start=(ft == 0), stop=(ft == 1),
                        )
                    if tail and ec % 2 == 1:
                        nc.scalar.copy(
                            out=osb[:, ec * 512:(ec + 1) * 512], in_=op)
                    else:
                        nc.vector.tensor_copy(
                            out=osb[:, ec * 512:(ec + 1) * 512], in_=op)
                    if tail and ec == 1:
                        # start the first half of the output write early so
                        # the end-of-program DMA drain is shorter
                        nc.sync.dma_start(out=out[b, rows, 0:1024],
                                          in_=osb[:, 0:1024])
                if tail:
                    nc.sync.dma_start(out=out[b, rows, 1024:2048],
                                      in_=osb[:, 1024:2048])
                else:
                    nc.sync.dma_start(out=out[b, rows, :], in_=osb)

            # ---- software-pipelined emission schedule ----
            for ck in range(4):
                emit_proj_dma(ck)
            emit_proj(0, startup=True)
            emit_late_consts()
            for ck in range(1, 8):
                if ck + 3 < 8:
                    emit_proj_dma(ck + 3)
                emit_proj(ck, startup=True)
            emit_k2_repl(0)

            proj_pending = list(range(8, 16))
            op_pending = []
            att_blocks = [(b, qh, g) for b in range(B) for qh in range(2)
                          for g in range(QH_PER_CORE)]
            ctx_queue = []
            for i, (b, qh, g) in enumerate(att_blocks):
                if proj_pending and proj_pending[0] not in stage_map:
                    emit_proj_dma(proj_pending[0])
                emit_att_sc(b, qh, g)
                ctx_queue.append((b, qh, g))
                if len(ctx_queue) > 2:
                    done = ctx_queue.pop(0)
                    emit_att_ctx(*done)
                    if done[2] == QH_PER_CORE - 1:
                        op_pending.extend((done[0], done[1], qq) for qq in range(8))
                if i == 7:
                    # b=1 attention needs all projections + its K replica
                    while proj_pending:
                        emit_proj(proj_pending.pop(0))
                    emit_k2_repl(1)
                    continue
                if proj_pending:
                    emit_proj(proj_pending.pop(0))
                    n_op = 1
                else:
                    n_op = 3
                for _ in range(min(n_op, len(op_pending))):
                    emit_oproj(*op_pending.pop(0))
            for done in ctx_queue:
                emit_att_ctx(*done)
                if done[2] == QH_PER_CORE - 1:
                    op_pending.extend((done[0], done[1], qq) for qq in range(8))
            while op_pending:
                emit_oproj(*op_pending.pop(0), tail=True)
    nc.compile()
    return nc


def kernel(hidden_states, attention_mask, Wq, bq, Wk, bk, Wv, bv, Wo, bo):
    hidden_states = np.asarray(hidden_states, dtype=np.float32)
    attention_mask = np.asarray(attention_mask, dtype=np.float32)
    Wq = np.asarray(Wq, dtype=np.float32)
    Wk = np.asarray(Wk, dtype=np.float32)
    Wv = np.asarray(Wv, dtype=np.float32)
    Wo = np.asarray(Wo, dtype=np.float32)

    if "nc" not in _CACHE:
        _CACHE["nc"] = _build_program()
    nc = _CACHE["nc"]

    hsT = np.ascontiguousarray(
        hidden_states.reshape(TOK, H).T).astype(ml_dtypes.bfloat16)  # [H, B*S]
    maskp = np.ascontiguousarray(
        attention_mask.reshape(B, S // 128, 128).transpose(2, 0, 1))  # [128, B, 16]
    sb16 = A16 * (maskp + EXP_SHIFT) + B16CONST
    prep = np.stack([maskp + EXP_SHIFT, sb16, -sb16], axis=-1).astype(np.float32)
    prep = np.ascontiguousarray(prep)                     # [128, B, 16, 3]
    eye = np.eye(128, dtype=np.float32).astype(ml_dtypes.bfloat16)

    in_maps = []
    for c in range(N_CORES):
        wq = Wq[QF * c:QF * (c + 1)]          # [256, H]
        wk = Wk[D * c:D * (c + 1)]            # [64, H]
        wv = Wv[D * c:D * (c + 1)]            # [64, H]
        wqkvT = np.ascontiguousarray(
            np.concatenate([wq, wk, wv], axis=0).T).astype(ml_dtypes.bfloat16)
        woT = np.ascontiguousarray(
            Wo[:, QF * c:QF * (c + 1)].T).astype(ml_dtypes.bfloat16)          # [256, H]
        bq_c = bq[QF * c:QF * (c + 1)] * ALPHA
        bk_c = bk[D * c:D * (c + 1)] * BETA
        bv_c = bv[D * c:D * (c + 1)]
        bqkv_c = np.ascontiguousarray(
            np.concatenate([bq_c, bk_c, bv_c]).astype(np.float32)
            .reshape(3, 128).T)               # [128, 3]
        in_maps.append({
            "hsT": hsT, "wqkvT": wqkvT, "woT": woT,
            "bqkv": bqkv_c, "prep": prep, "eye": eye,
        })

    _CACHE["last_in_maps"] = in_maps
    res = bass_utils.run_bass_kernel_spmd(nc, in_maps, core_ids=list(range(N_CORES)))
    acc = np.zeros((B, S, H), dtype=np.float32)
    for c in range(N_CORES):
        acc += np.asarray(res.results[c]["out"], dtype=np.float32)
    acc += np.asarray(bo, dtype=np.float32)[None, None, :]
    return acc
